# revision 36
# baseline (speedup 1.0000x reference)
"""DCNv4-1D fused Trainium2 kernel v2. Data-parallel over batch N, 8 cores.

Per core (one sample), redesigned against the TimelineSim cost model:
  - x loaded via SWDGE cast-DMA f32->f16 (cost model charges OUT bytes).
  - LN stats: f16 ones-matmuls on PE into PSUM [33,512] (sum row 0, sumsq
    row 32); ACT copies to an f16 row; tiny DMAs repack to [128,w]; DVE/ACT
    finish rs/murs; DRAM round-trip broadcasts rs|murs to [128, 2*SC] f16.
    All Sqrt before any Gelu (2 ACT table loads total).
  - Apply: v = x16*rs_b, u = v - murs_b (DVE f16 TT, 2x mode), then
    xa = gelu(lnw*u + lnb) on ACT into xa_full with zeroed halo guards.
  - om = om_wT' @ xa on PE (f16), ACT adds bias(+grid fold), scatter DMAs
    land rows in qm_r [(g,s16), (o,f)] layout.
  - Banded conv (5 diagonals d=-2..2): coeff on DVE (tensor_scalar 4x via
    abs_max trick + TT mask/ksum); products coeff*xa on DVE f16 TT;
    d-sum mostly via PE id16 matmuls into PSUM + ACT copy-out, partly as
    DVE TT adds writing f16 directly (knob).
  - L split in 2 halves: half-0 conv overlaps half-1 front-end.
"""

import json

import numpy as np

N, C, L = 8, 256, 8192
G, K, GC = 8, 3, 32
LN_EPS = 1e-6
NCT = 2
NHALF = 2
LH = L // NHALF          # 4096
SC = 2048                # stats superchunk
NSC = L // SC            # 4 (2 per half)
SW = SC // 128           # 16 stats cols per partition per sc
CH = 256                 # conv chunk (16 per half -> partitions (g, s))
NCH_H = LH // CH         # 16
LQ = CH
HALO = 4
WIN = CH + 2 * HALO      # 264
D_LO, D_HI = -2, 2
ND = D_HI - D_LO + 1     # 5
CPG = 4                  # channels per product op
NCP = GC // CPG          # 4 product blocks per half

# ---- tuning knobs ----
DSUM_DVE_CP = ()         # cp blocks whose d-sum runs on DVE (rest PE+ACT)
OST_POOL_JP = ()         # (cp, jp) pairs whose PSUM->f16 copy runs on Pool
SCAT_SWDGE = True        # half the qm scatters on SWDGE (Pool) vs HWDGE

_cache = {}
_DBG_PSUM_INIT = False
_DBG_RELU = False


# --- BIR post-pass: this walrus build rejects >1 sync wait per instruction;
# split extras onto same-engine NoOps inserted just before the owner. ---
def _split_multi_waits(bir_json: bytes, max_waits: int = 1) -> bytes:
    j = json.loads(bir_json)
    n = [0]

    def fresh():
        n[0] += 1
        return f"I-wsplit-{n[0]}"

    for fn in j.get("functions", []):
        for bb in fn.get("basicblocks", []) or fn.get("blocks", []) or []:
            out = []
            for inst in bb.get("instructions", []):
                si = inst.get("sync_info")
                waits = (si or {}).get("on_wait") or []
                if len(waits) > max_waits:
                    for w in waits[:-max_waits]:
                        out.append({
                            "debug": inst.get("debug", 0),
                            "engine": inst["engine"],
                            "ins": [], "outs": [],
                            "name": fresh(),
                            "opcode": "NoOp",
                            "sync_info": {"on_update": [], "on_wait": [w]},
                        })
                    si["on_wait"] = waits[-max_waits:]
                out.append(inst)
            bb["instructions"] = out
    return json.dumps(j).encode()


def _install_patch():
    import os

    # The elide-DMA sem optimization assumes per-ring FIFO but DMAHW lanes
    # are shared across the SP/ACT rings; with our cross-ring interleave it
    # miscounts and consumers fire early. Disable it.
    os.environ["BACC_ELIDE_DMA_OPT_LIMIT"] = "0"
    import concourse.tile_sem_assignment as _tsa
    _tsa._opt_limit = 0

    import concourse.bass2jax as bass2jax
    import concourse.bass_utils as bass_utils

    if getattr(bass2jax.compile_bir_kernel, "_wsplit", False):
        return
    orig = bass_utils.compile_bir_kernel

    def patched(bir_json, tmpdir, neff_name="file.neff"):
        return orig(_split_multi_waits(bir_json), tmpdir, neff_name=neff_name)

    patched._wsplit = True
    bass_utils.compile_bir_kernel = patched
    bass2jax.compile_bir_kernel = patched


def _build_module():
    import contextlib

    import concourse.bass as bass
    import concourse.tile as tile
    from concourse import mybir

    f32 = mybir.dt.float32
    f16 = mybir.dt.float16
    AF = mybir.ActivationFunctionType
    GELU = AF.Relu if _DBG_RELU else AF.Gelu
    OP = mybir.AluOpType

    nc = bass.Bass()

    x_d = nc.dram_tensor("x", [C, L], f32, kind="ExternalInput")
    cst32_d = nc.dram_tensor("cst32", [128, 6], f32, kind="ExternalInput")
    cst16_d = nc.dram_tensor("cst16", [128, 226], f16, kind="ExternalInput")
    scr_d = nc.dram_tensor("scr", [NSC, 2 * SC], f16, kind="Internal")
    out_d = nc.dram_tensor("out", [C, L], f16, kind="ExternalOutput")

    with tile.TileContext(nc) as tc, contextlib.ExitStack() as ctx:
        const = ctx.enter_context(tc.tile_pool(name="const", bufs=1))
        persist = ctx.enter_context(tc.tile_pool(name="persist", bufs=1))
        xqp = ctx.enter_context(tc.tile_pool(name="xq", bufs=1))
        srp = ctx.enter_context(tc.tile_pool(name="srp", bufs=2))
        stp = ctx.enter_context(tc.tile_pool(name="stp", bufs=2))
        rbp = ctx.enter_context(tc.tile_pool(name="rbp", bufs=3))
        vup = ctx.enter_context(tc.tile_pool(name="vup", bufs=2))
        omp = ctx.enter_context(tc.tile_pool(name="omp", bufs=2))
        xrp = ctx.enter_context(tc.tile_pool(name="xrp", bufs=2))
        qmp = ctx.enter_context(tc.tile_pool(name="qmp", bufs=2))
        cfp = ctx.enter_context(tc.tile_pool(name="cfp", bufs=1))
        tmpp = ctx.enter_context(tc.tile_pool(name="tmpp", bufs=2))
        outp = ctx.enter_context(tc.tile_pool(name="outp", bufs=1))
        psum = ctx.enter_context(tc.tile_pool(name="psum", bufs=2,
                                              space="PSUM"))
        psum_o = ctx.enter_context(tc.tile_pool(name="psum_o", bufs=2,
                                                space="PSUM"))
        psum_a = ctx.enter_context(tc.tile_pool(name="psum_a", bufs=4,
                                                space="PSUM"))

        # HWDGE lane purity: DMAHW lanes are assigned round-robin over ALL
        # hwdge DMAs; threshold waits are only sound if each lane is fed by
        # one FIFO ring. Strict SP/ACT alternation keeps even lanes SP-only
        # and odd lanes ACT-only (NUM_HWDGE_SEMS == 8).
        _hwflip = [0]

        def hw_dma(out, in_):
            eng = nc.sync if _hwflip[0] % 2 == 0 else nc.scalar
            _hwflip[0] += 1
            eng.dma_start(out=out, in_=in_)

        # ---------------- constants ----------------
        cst32 = const.tile([128, 6], f32, tag="cst32", name="cst32")
        hw_dma(out=cst32, in_=cst32_d[:])
        cst16 = const.tile([128, 226], f16, tag="cst16", name="cst16")
        hw_dma(out=cst16, in_=cst16_d[:])
        lnw_c = [cst32[:, ct:ct + 1] for ct in range(NCT)]
        lnb_c = [cst32[:, 2 + ct:3 + ct] for ct in range(NCT)]
        bias48 = cst32[0:48, 5:6]
        omwT = [cst16[:, ct * 48:(ct + 1) * 48] for ct in range(NCT)]
        id16 = cst16[:, 96:224]
        onesc16 = cst16[:, 224:225]
        eps_c = const.tile([128, 1], f32, tag="eps", name="eps")
        nc.vector.memset(eps_c, LN_EPS)

        # ---------------- persistent tensors ----------------
        x16 = [persist.tile([128, L], f16, tag=f"x16_{ct}",
                            name=f"x16_{ct}") for ct in range(NCT)]
        xa_full = [persist.tile([128, 2 * HALO + L], f16, tag=f"xaf{ct}",
                                name=f"xaf{ct}") for ct in range(NCT)]
        for ct in range(NCT):
            nc.vector.memset(xa_full[ct][:, 0:HALO], 0.0)
            nc.vector.memset(xa_full[ct][:, HALO + L:2 * HALO + L], 0.0)

        # x cast loads (SWDGE), chunked per superchunk so stats start early
        for sc in range(NSC):
            for ct in range(NCT):
                nc.gpsimd.dma_start(
                    out=x16[ct][:, sc * SC:(sc + 1) * SC],
                    in_=x_d[ct * 128:(ct + 1) * 128, sc * SC:(sc + 1) * SC])

        rb_l = [None] * NSC

        # ============ phase 1: LN stats, staged across superchunks ============
        # (same-stage DMAs are adjacent in each queue so the four superchunk
        # round-trips pipeline instead of serializing end-to-end)
        spk_l, qpk_l, rsmu_l = [], [], []
        for sc in range(NSC):
            lo = sc * SC
            srow = srp.tile([33, SC], f16, tag="srow", name="srow")
            xq = [None] * NCT
            for ct in range(NCT):
                xq[ct] = xqp.tile([128, SC], f16, tag=f"xq{ct}",
                                  name=f"xq{ct}")
                nc.vector.tensor_tensor(out=xq[ct],
                                        in0=x16[ct][:, lo:lo + SC],
                                        in1=x16[ct][:, lo:lo + SC],
                                        op=OP.mult)
            for c in range(SC // 512):
                cf = c * 512
                st_ps = psum.tile([33, 512], f32, tag="st", name="st")
                if _DBG_PSUM_INIT:
                    nc.vector.memset(st_ps[1:32, :], 0.0)
                for ct in range(NCT):
                    nc.tensor.matmul(st_ps[0:1, :], onesc16,
                                     x16[ct][:, lo + cf:lo + cf + 512],
                                     start=(ct == 0), stop=(ct == NCT - 1))
                for ct in range(NCT):
                    nc.tensor.matmul(st_ps[32:33, :], onesc16,
                                     xq[ct][:, cf:cf + 512],
                                     start=(ct == 0), stop=(ct == NCT - 1))
                nc.scalar.copy(out=srow[:, cf:cf + 512], in_=st_ps)

            spk = stp.tile([128, SW], f16, tag=f"spk{sc}", name=f"spk{sc}")
            hw_dma(
                out=spk,
                in_=srow[0:1, :].rearrange("one (p w) -> one p w", p=128))
            qpk = stp.tile([128, SW], f16, tag=f"qpk{sc}", name=f"qpk{sc}")
            hw_dma(
                out=qpk,
                in_=srow[32:33, :].rearrange("one (p w) -> one p w", p=128))
            spk_l.append(spk)
            qpk_l.append(qpk)

        for sc in range(NSC):
            spk, qpk = spk_l[sc], qpk_l[sc]
            mu_t = stp.tile([128, SW], f32, tag="mu_t", name="mu_t")
            nc.vector.tensor_scalar_mul(out=mu_t, in0=spk, scalar1=1.0 / C)
            musq = stp.tile([128, SW], f32, tag="musq", name="musq")
            nc.vector.tensor_tensor(out=musq, in0=mu_t, in1=mu_t,
                                    op=OP.mult)
            varq = stp.tile([128, SW], f32, tag="varq", name="varq")
            nc.vector.scalar_tensor_tensor(out=varq, in0=qpk, scalar=1.0 / C,
                                           in1=musq, op0=OP.mult,
                                           op1=OP.subtract)
            sd = stp.tile([128, SW], f32, tag="sd", name="sd")
            nc.scalar.activation(out=sd, in_=varq, func=AF.Sqrt,
                                 bias=eps_c, scale=1.0)
            rs32 = stp.tile([128, SW], f32, tag="rs32", name="rs32")
            nc.vector.reciprocal(out=rs32, in_=sd)
            rsmu = stp.tile([128, 2 * SW], f16, tag=f"rsmu{sc}",
                            name=f"rsmu{sc}")
            nc.vector.tensor_scalar_mul(out=rsmu[:, 0:SW], in0=rs32,
                                        scalar1=1.0)
            nc.vector.scalar_tensor_tensor(out=rsmu[:, SW:2 * SW], in0=spk,
                                           scalar=1.0 / C, in1=rs32,
                                           op0=OP.mult, op1=OP.mult)
            rsmu_l.append(rsmu)
            nc.gpsimd.dma_start(
                out=scr_d[sc:sc + 1, :].rearrange("one (p w) -> one p w",
                                                  p=128),
                in_=rsmu)

        for sc in range(NSC):
            rb = rbp.tile([128, 2 * SC], f16, tag="rb", name="rb")
            row = scr_d[sc:sc + 1, :]
            hw_dma(
                out=rb,
                in_=bass.AP(tensor=row.tensor, offset=row.offset,
                            ap=[[0, 128], [1, 2 * SC]]))
            rb_l[sc] = rb

        # per-half state
        qm_r = [None] * NHALF
        xa_r = [None] * NHALF
        prev_hi = {0: -1, 1: -1}

        def windows_dma(h, s, ct, dma):
            # xa window for conv chunk s of half h, channel tile ct
            st = h * LH + s * CH
            dma(
                out=xa_r[h][ct * 64 + s:ct * 64 + s + 49:16, :]
                .rearrange("g (cc w) -> g cc w", cc=GC),
                in_=xa_full[ct][:, st:st + WIN])

        def flush_windows(h, s_hi):
            s_hi = min(s_hi, NCH_H - 1)
            for s in range(prev_hi[h] + 1, s_hi + 1):
                for ct in range(NCT):
                    eng = hw_dma if (s + ct) % 2 == 0 else nc.gpsimd.dma_start
                    windows_dma(h, s, ct, eng)
            prev_hi[h] = max(prev_hi[h], s_hi)

        # ============= phase 2: apply + gelu + om (per half) =============
        APW = 1024  # apply op width
        SC_PER_H = LH // SC  # 2

        def emit_apply(sc, lo_off, width):
            # LN apply + gelu for x columns [sc*SC+lo_off, +width)
            rb_v = rb_l[sc][:].rearrange("q (p half w) -> q p half w",
                                         p=128, half=2)
            rs_sl = rb_v[:, :, 0, :]
            mu_sl = rb_v[:, :, 1, :]
            lo = sc * SC + lo_off
            pw = width // SW
            p0 = lo_off // SW
            for ct in range(NCT):
                xs = x16[ct][:, lo:lo + width].rearrange(
                    "p (a w) -> p a w", a=pw)
                v = vup.tile([128, APW], f16, tag=f"v{ct}", name=f"v{ct}")
                vv = v[:, 0:width].rearrange("p (a w) -> p a w", a=pw)
                nc.vector.tensor_tensor(
                    out=vv, in0=xs, in1=rs_sl[:, p0:p0 + pw, :], op=OP.mult)
                nc.vector.tensor_tensor(
                    out=vv, in0=vv, in1=mu_sl[:, p0:p0 + pw, :],
                    op=OP.subtract)
                nc.scalar.activation(
                    out=xa_full[ct][:, HALO + lo:HALO + lo + width],
                    in_=v[:, 0:width], func=GELU, bias=lnb_c[ct],
                    scale=lnw_c[ct])

        def emit_ph2(h):
            qm_r[h] = qmp.tile([128, 2 * K * LQ], f16, tag="qm_r",
                               name=f"qm_r{h}")
            xa_r[h] = xrp.tile([128, GC * WIN], f16, tag="xa_r",
                               name=f"xa_r{h}")
            for sc_h in range(SC_PER_H):
                sc = h * SC_PER_H + sc_h
                for a2 in range(SC // APW):
                    emit_apply(sc, a2 * APW, APW)
                    # om for the two 512-chunks inside this apply window
                    for g2 in range(APW // 512):
                        gp = sc_h * (SC // 512) + a2 * 2 + g2
                        lo5 = h * LH + gp * 512
                        om_ps = psum_o.tile([48, 512], f32, tag="om",
                                            name="om")
                        for ct in range(NCT):
                            nc.tensor.matmul(
                                om_ps, omwT[ct],
                                xa_full[ct][:, HALO + lo5:HALO + lo5 + 512],
                                start=(ct == 0), stop=(ct == NCT - 1))
                        om_st = omp.tile([48, 512], f16, tag="om_st",
                                         name="om_st")
                        nc.scalar.activation(out=om_st, in_=om_ps,
                                             func=AF.Identity, bias=bias48,
                                             scale=1.0)
                        for s2 in range(2):
                            s = gp * 2 + s2  # conv chunk within half
                            hw_dma(
                                out=qm_r[h][s:128:16, :]
                                .rearrange("g (o f) -> g o f", o=6),
                                in_=om_st[:, s2 * 256:(s2 + 1) * 256])
                        # windows whose xa data is now fully written:
                        # chunk s needs xa through local l = s*CH + CH+HALO-1
                        flush_windows(h, ((gp + 1) * 512 - CH - HALO) // CH)
            if h == 0:
                # pre-compute h1's first 512 cols so h0's last window (and
                # thus h0's conv) doesn't wait for h1's front-end. h1
                # recomputes the same values later (idempotent).
                emit_apply(SC_PER_H, 0, 512)
            flush_windows(h, NCH_H - 1)

        # ================= phase 3: banded conv per half =================
        def emit_ph3(h):
            q_ap = qm_r[h][:, 0:K * LQ]
            m_ap = qm_r[h][:, K * LQ:2 * K * LQ]
            hm_all = cfp.tile([128, ND * K * LQ], f16, tag="hm",
                              name=f"hm{h}")
            r2t = cfp.tile([128, K * LQ], f16, tag="r2t", name="r2t")
            for i in range(ND):
                d = D_LO + i
                sl = hm_all[:, i * K * LQ:(i + 1) * K * LQ]
                nc.vector.tensor_scalar_add(out=sl, in0=q_ap,
                                            scalar1=float(1 - d))
                nc.vector.tensor_scalar(out=r2t, in0=q_ap,
                                        scalar1=float(d + 1), scalar2=-1.0,
                                        op0=OP.subtract, op1=OP.mult)
                nc.vector.tensor_tensor(out=sl, in0=sl, in1=r2t, op=OP.min)
                nc.vector.tensor_scalar_max(out=sl, in0=sl, scalar1=0.0)
            m_rep = bass.AP(tensor=m_ap.tensor, offset=m_ap.offset,
                            ap=[m_ap.ap[0], [0, ND], [1, K * LQ]])
            nc.vector.tensor_tensor(
                out=hm_all[:].rearrange("p (d f) -> p d f", d=ND),
                in0=hm_all[:].rearrange("p (d f) -> p d f", d=ND),
                in1=m_rep, op=OP.mult)
            cd_all = cfp.tile([128, ND * LQ], f16, tag="cd", name=f"cd{h}")
            hm_v = hm_all[:].rearrange("p (d k f) -> p d k f", d=ND, k=K)
            cd_v = cd_all[:].rearrange("p (d f) -> p d f", d=ND)
            nc.vector.tensor_tensor(out=cd_v, in0=hm_v[:, :, 0, :],
                                    in1=hm_v[:, :, 1, :], op=OP.add)
            nc.vector.tensor_tensor(out=cd_v, in0=cd_v,
                                    in1=hm_v[:, :, 2, :], op=OP.add)

            xa_r_v = xa_r[h][:].rearrange("p (c wn) -> p c wn", c=GC)
            out_t = out_d[:]
            ost_all = outp.tile([128, GC * LQ], f16, tag="ost_all",
                                name=f"ost_all{h}")
            for cp in range(NCP):
                tmps = []
                for i in range(ND):
                    d = D_LO + i
                    tmp = tmpp.tile([128, CPG * LQ], f16, tag=f"tmp{i}",
                                    name=f"tmp{i}")
                    cdi = cd_all[:, i * LQ:(i + 1) * LQ]
                    cb = bass.AP(tensor=cdi.tensor, offset=cdi.offset,
                                 ap=[cdi.ap[0], [0, CPG], [1, LQ]])
                    nc.vector.tensor_tensor(
                        out=tmp[:].rearrange("p (c f) -> p c f", c=CPG),
                        in0=xa_r_v[:, cp * CPG:(cp + 1) * CPG,
                                   HALO + d:HALO + d + LQ],
                        in1=cb, op=OP.mult)
                    tmps.append(tmp)
                o0 = cp * CPG * LQ
                if cp in DSUM_DVE_CP:
                    nc.vector.tensor_tensor(out=tmps[0], in0=tmps[0],
                                            in1=tmps[1], op=OP.add)
                    nc.vector.tensor_tensor(out=tmps[2], in0=tmps[2],
                                            in1=tmps[3], op=OP.add)
                    nc.vector.tensor_tensor(out=tmps[0], in0=tmps[0],
                                            in1=tmps[2], op=OP.add)
                    nc.vector.tensor_tensor(
                        out=ost_all[:, o0:o0 + CPG * LQ], in0=tmps[0],
                        in1=tmps[4], op=OP.add)
                else:
                    for jp in range(CPG // 2):
                        acc = psum_a.tile([128, 2 * LQ], f32, tag="acc",
                                          name="acc")
                        for jj in range(2):
                            j = jp * 2 + jj
                            for n_i in range(ND):
                                nc.tensor.matmul(
                                    acc[:, jj * LQ:(jj + 1) * LQ], id16,
                                    tmps[n_i][:, j * LQ:(j + 1) * LQ],
                                    start=(n_i == 0), stop=(n_i == ND - 1))
                        oj = o0 + jp * 2 * LQ
                        if (cp, jp) in OST_POOL_JP:
                            nc.gpsimd.tensor_copy(
                                out=ost_all[:, oj:oj + 2 * LQ], in_=acc)
                        else:
                            nc.scalar.copy(
                                out=ost_all[:, oj:oj + 2 * LQ], in_=acc)
            for g in range(G):
                hw_dma(
                    out=bass.AP(
                        tensor=out_t.tensor,
                        offset=out_t.offset + g * GC * L + h * LH,
                        ap=[[CH, NCH_H], [L, GC], [1, LQ]]),
                    in_=ost_all[g * 16:g * 16 + 16, :]
                    .rearrange("s (c f) -> s c f", c=GC))

        # driver: front-ends first (conv overlaps them via per-engine
        # in-order streams)
        emit_ph2(0)
        emit_ph2(1)
        emit_ph3(0)
        emit_ph3(1)

    return nc


def _prep_params(ln_w, ln_b, om_w, om_b):
    cst32 = np.zeros((128, 6), np.float32)
    cst32[:, 0] = ln_w[0:128]
    cst32[:, 1] = ln_w[128:256]
    cst32[:, 2] = ln_b[0:128]
    cst32[:, 3] = ln_b[128:256]
    cst32[:, 4] = 1.0
    # bias48 in (g, qm, k) order with conv grid fold for qm=0 (offsets)
    for g in range(G):
        for k in range(K):
            cst32[g * 6 + k, 5] = om_b[g * K + k] + (k - 1.0)
            cst32[g * 6 + 3 + k, 5] = om_b[G * K + g * K + k]
    cst16 = np.zeros((128, 226), np.float16)
    omwT = om_w.T.astype(np.float16)          # [C, 48] original row order
    perm = np.zeros(48, np.int64)
    for g in range(G):
        for k in range(K):
            perm[g * 6 + k] = g * K + k
            perm[g * 6 + 3 + k] = G * K + g * K + k
    omwTp = omwT[:, perm]                      # columns in (g,qm,k) order
    cst16[:, 0:48] = omwTp[0:128]
    cst16[:, 48:96] = omwTp[128:256]
    cst16[:, 96:224] = np.eye(128, dtype=np.float16)
    cst16[:, 224] = 1.0
    return {"cst32": cst32, "cst16": cst16}


def kernel(x, ln_w, ln_b, om_w, om_b):
    _install_patch()
    from concourse.bass_utils import run_bass_kernel_spmd

    if "nc" not in _cache:
        _cache["nc"] = _build_module()
    nc = _cache["nc"]

    x = np.ascontiguousarray(np.asarray(x, dtype=np.float32))
    params = _prep_params(np.asarray(ln_w, np.float32),
                          np.asarray(ln_b, np.float32),
                          np.asarray(om_w, np.float32),
                          np.asarray(om_b, np.float32))
    in_maps = [dict(params, x=x[n]) for n in range(N)]
    res = run_bass_kernel_spmd(nc, in_maps, core_ids=list(range(N)))
    return np.stack([res.results[n]["out"] for n in range(N)],
                    axis=0).astype(np.float32)


def _prep_inputs(inputs):
    x = np.ascontiguousarray(np.asarray(inputs["x"], dtype=np.float32))
    params = _prep_params(np.asarray(inputs["ln_w"], np.float32),
                          np.asarray(inputs["ln_b"], np.float32),
                          np.asarray(inputs["om_w"], np.float32),
                          np.asarray(inputs["om_b"], np.float32))
    return [dict(params, x=x[n]) for n in range(N)]


def run_traced(inputs):
    _install_patch()
    from concourse.bass_utils import run_bass_kernel_spmd
    if "nc" not in _cache:
        _cache["nc"] = _build_module()
    return run_bass_kernel_spmd(_cache["nc"], _prep_inputs(inputs),
                                core_ids=list(range(N)), trace=True)


# revision 44
# speedup vs baseline: 1.0494x; 1.0494x over previous
"""DCNv4-1D fused Trainium2 kernel v2. Data-parallel over batch N, 8 cores.

Per core (one sample), redesigned against the TimelineSim cost model:
  - x loaded via SWDGE cast-DMA f32->f16 (cost model charges OUT bytes).
  - LN stats: f16 ones-matmuls on PE into PSUM [33,512] (sum row 0, sumsq
    row 32); ACT copies to an f16 row; tiny DMAs repack to [128,w]; DVE/ACT
    finish rs/murs; DRAM round-trip broadcasts rs|murs to [128, 2*SC] f16.
    All Sqrt before any Gelu (2 ACT table loads total).
  - Apply: v = x16*rs_b, u = v - murs_b (DVE f16 TT, 2x mode), then
    xa = gelu(lnw*u + lnb) on ACT into xa_full with zeroed halo guards.
  - om = om_wT' @ xa on PE (f16), ACT adds bias(+grid fold), scatter DMAs
    land rows in qm_r [(g,s16), (o,f)] layout.
  - Banded conv (5 diagonals d=-2..2): coeff on DVE (tensor_scalar 4x via
    abs_max trick + TT mask/ksum); products coeff*xa on DVE f16 TT;
    d-sum mostly via PE id16 matmuls into PSUM + ACT copy-out, partly as
    DVE TT adds writing f16 directly (knob).
  - L split in 2 halves: half-0 conv overlaps half-1 front-end.
"""

import json

import numpy as np

N, C, L = 8, 256, 8192
G, K, GC = 8, 3, 32
LN_EPS = 1e-6
NCT = 2
NHALF = 2
LH = L // NHALF          # 4096
SC = 2048                # stats superchunk
NSC = L // SC            # 4 (2 per half)
SW = SC // 128           # 16 stats cols per partition per sc
CH = 256                 # conv chunk (16 per half -> partitions (g, s))
NCH_H = LH // CH         # 16
LQ = CH
HALO = 4
WIN = CH + 2 * HALO      # 264
D_LO, D_HI = -2, 2
ND = D_HI - D_LO + 1     # 5
CPG = 4                  # channels per product op
NCP = GC // CPG          # 4 product blocks per half

# ---- tuning knobs ----
DSUM_DVE_CP = ()         # cp blocks whose d-sum runs on DVE (rest PE+ACT)
OST_POOL_JP = ()         # (cp, jp) pairs whose PSUM->f16 copy runs on Pool
SCAT_SWDGE = True        # half the qm scatters on SWDGE (Pool) vs HWDGE

_cache = {}
_DBG_PSUM_INIT = False
_DBG_RELU = False


# --- BIR post-pass: this walrus build rejects >1 sync wait per instruction;
# split extras onto same-engine NoOps inserted just before the owner. ---
def _split_multi_waits(bir_json: bytes, max_waits: int = 1) -> bytes:
    j = json.loads(bir_json)
    n = [0]

    def fresh():
        n[0] += 1
        return f"I-wsplit-{n[0]}"

    for fn in j.get("functions", []):
        for bb in fn.get("basicblocks", []) or fn.get("blocks", []) or []:
            out = []
            for inst in bb.get("instructions", []):
                si = inst.get("sync_info")
                waits = (si or {}).get("on_wait") or []
                if len(waits) > max_waits:
                    for w in waits[:-max_waits]:
                        out.append({
                            "debug": inst.get("debug", 0),
                            "engine": inst["engine"],
                            "ins": [], "outs": [],
                            "name": fresh(),
                            "opcode": "NoOp",
                            "sync_info": {"on_update": [], "on_wait": [w]},
                        })
                    si["on_wait"] = waits[-max_waits:]
                out.append(inst)
            bb["instructions"] = out
    return json.dumps(j).encode()


def _install_patch():
    import os

    # The elide-DMA sem optimization assumes per-ring FIFO but DMAHW lanes
    # are shared across the SP/ACT rings; with our cross-ring interleave it
    # miscounts and consumers fire early. Disable it.
    os.environ["BACC_ELIDE_DMA_OPT_LIMIT"] = "0"
    import concourse.tile_sem_assignment as _tsa
    _tsa._opt_limit = 0

    import concourse.bass2jax as bass2jax
    import concourse.bass_utils as bass_utils

    if getattr(bass2jax.compile_bir_kernel, "_wsplit", False):
        return
    orig = bass_utils.compile_bir_kernel

    def patched(bir_json, tmpdir, neff_name="file.neff"):
        return orig(_split_multi_waits(bir_json), tmpdir, neff_name=neff_name)

    patched._wsplit = True
    bass_utils.compile_bir_kernel = patched
    bass2jax.compile_bir_kernel = patched


def _build_module():
    import contextlib

    import concourse.bass as bass
    import concourse.tile as tile
    from concourse import mybir

    f32 = mybir.dt.float32
    f16 = mybir.dt.float16
    AF = mybir.ActivationFunctionType
    GELU = AF.Relu if _DBG_RELU else AF.Gelu
    OP = mybir.AluOpType

    nc = bass.Bass()

    x_d = nc.dram_tensor("x", [C, L], f32, kind="ExternalInput")
    cst32_d = nc.dram_tensor("cst32", [128, 6], f32, kind="ExternalInput")
    cst16_d = nc.dram_tensor("cst16", [128, 226], f16, kind="ExternalInput")
    scr_d = nc.dram_tensor("scr", [NSC, 2 * SC], f16, kind="Internal")
    out_d = nc.dram_tensor("out", [C, L], f16, kind="ExternalOutput")

    with tile.TileContext(nc) as tc, contextlib.ExitStack() as ctx:
        const = ctx.enter_context(tc.tile_pool(name="const", bufs=1))
        persist = ctx.enter_context(tc.tile_pool(name="persist", bufs=1))
        xqp = ctx.enter_context(tc.tile_pool(name="xq", bufs=1))
        srp = ctx.enter_context(tc.tile_pool(name="srp", bufs=2))
        stp = ctx.enter_context(tc.tile_pool(name="stp", bufs=2))
        rbp = ctx.enter_context(tc.tile_pool(name="rbp", bufs=3))
        vup = ctx.enter_context(tc.tile_pool(name="vup", bufs=2))
        omp = ctx.enter_context(tc.tile_pool(name="omp", bufs=2))
        xrp = ctx.enter_context(tc.tile_pool(name="xrp", bufs=2))
        qmp = ctx.enter_context(tc.tile_pool(name="qmp", bufs=2))
        cfp = ctx.enter_context(tc.tile_pool(name="cfp", bufs=1))
        tmpp = ctx.enter_context(tc.tile_pool(name="tmpp", bufs=2))
        outp = ctx.enter_context(tc.tile_pool(name="outp", bufs=1))
        psum = ctx.enter_context(tc.tile_pool(name="psum", bufs=2,
                                              space="PSUM"))
        psum_o = ctx.enter_context(tc.tile_pool(name="psum_o", bufs=2,
                                                space="PSUM"))
        psum_a = ctx.enter_context(tc.tile_pool(name="psum_a", bufs=4,
                                                space="PSUM"))

        # HWDGE lane purity: DMAHW lanes are assigned round-robin over ALL
        # hwdge DMAs; threshold waits are only sound if each lane is fed by
        # one FIFO ring. Strict SP/ACT alternation keeps even lanes SP-only
        # and odd lanes ACT-only (NUM_HWDGE_SEMS == 8).
        _hwflip = [0]

        def hw_dma(out, in_):
            eng = nc.sync if _hwflip[0] % 2 == 0 else nc.scalar
            _hwflip[0] += 1
            eng.dma_start(out=out, in_=in_)

        # ---------------- constants ----------------
        cst32 = const.tile([128, 6], f32, tag="cst32", name="cst32")
        hw_dma(out=cst32, in_=cst32_d[:])
        cst16 = const.tile([128, 226], f16, tag="cst16", name="cst16")
        hw_dma(out=cst16, in_=cst16_d[:])
        lnw_c = [cst32[:, ct:ct + 1] for ct in range(NCT)]
        lnb_c = [cst32[:, 2 + ct:3 + ct] for ct in range(NCT)]
        bias48 = cst32[0:48, 5:6]
        omwT = [cst16[:, ct * 48:(ct + 1) * 48] for ct in range(NCT)]
        id16 = cst16[:, 96:224]
        onesc16 = cst16[:, 224:225]
        eps_c = const.tile([128, 1], f32, tag="eps", name="eps")
        nc.vector.memset(eps_c, LN_EPS)

        # ---------------- persistent tensors ----------------
        x16 = [persist.tile([128, L], f16, tag=f"x16_{ct}",
                            name=f"x16_{ct}") for ct in range(NCT)]
        xa_full = [persist.tile([128, 2 * HALO + L], f16, tag=f"xaf{ct}",
                                name=f"xaf{ct}") for ct in range(NCT)]
        for ct in range(NCT):
            nc.vector.memset(xa_full[ct][:, 0:HALO], 0.0)
            nc.vector.memset(xa_full[ct][:, HALO + L:2 * HALO + L], 0.0)

        # x cast loads (SWDGE), chunked per superchunk so stats start early
        for sc in range(NSC):
            for ct in range(NCT):
                nc.gpsimd.dma_start(
                    out=x16[ct][:, sc * SC:(sc + 1) * SC],
                    in_=x_d[ct * 128:(ct + 1) * 128, sc * SC:(sc + 1) * SC])

        rb_l = [None] * NSC

        # ============ phase 1: LN stats, staged across superchunks ============
        # (same-stage DMAs are adjacent in each queue so the four superchunk
        # round-trips pipeline instead of serializing end-to-end)
        spk_l, qpk_l, rsmu_l = [], [], []
        for sc in range(NSC):
            lo = sc * SC
            srow = srp.tile([33, SC], f16, tag="srow", name="srow")
            xq = [None] * NCT
            for ct in range(NCT):
                xq[ct] = xqp.tile([128, SC], f16, tag=f"xq{ct}",
                                  name=f"xq{ct}")
                nc.vector.tensor_tensor(out=xq[ct],
                                        in0=x16[ct][:, lo:lo + SC],
                                        in1=x16[ct][:, lo:lo + SC],
                                        op=OP.mult)
            for c in range(SC // 512):
                cf = c * 512
                st_ps = psum.tile([33, 512], f32, tag="st", name="st")
                if _DBG_PSUM_INIT:
                    nc.vector.memset(st_ps[1:32, :], 0.0)
                for ct in range(NCT):
                    nc.tensor.matmul(st_ps[0:1, :], onesc16,
                                     x16[ct][:, lo + cf:lo + cf + 512],
                                     start=(ct == 0), stop=(ct == NCT - 1))
                for ct in range(NCT):
                    nc.tensor.matmul(st_ps[32:33, :], onesc16,
                                     xq[ct][:, cf:cf + 512],
                                     start=(ct == 0), stop=(ct == NCT - 1))
                nc.scalar.copy(out=srow[:, cf:cf + 512], in_=st_ps)

            spk = stp.tile([128, SW], f16, tag=f"spk{sc}", name=f"spk{sc}")
            hw_dma(
                out=spk,
                in_=srow[0:1, :].rearrange("one (p w) -> one p w", p=128))
            qpk = stp.tile([128, SW], f16, tag=f"qpk{sc}", name=f"qpk{sc}")
            hw_dma(
                out=qpk,
                in_=srow[32:33, :].rearrange("one (p w) -> one p w", p=128))
            spk_l.append(spk)
            qpk_l.append(qpk)

        for sc in range(NSC):
            spk, qpk = spk_l[sc], qpk_l[sc]
            mu_t = stp.tile([128, SW], f32, tag="mu_t", name="mu_t")
            nc.vector.tensor_scalar_mul(out=mu_t, in0=spk, scalar1=1.0 / C)
            musq = stp.tile([128, SW], f32, tag="musq", name="musq")
            nc.vector.tensor_tensor(out=musq, in0=mu_t, in1=mu_t,
                                    op=OP.mult)
            varq = stp.tile([128, SW], f32, tag="varq", name="varq")
            nc.vector.scalar_tensor_tensor(out=varq, in0=qpk, scalar=1.0 / C,
                                           in1=musq, op0=OP.mult,
                                           op1=OP.subtract)
            sd = stp.tile([128, SW], f32, tag="sd", name="sd")
            nc.scalar.activation(out=sd, in_=varq, func=AF.Sqrt,
                                 bias=eps_c, scale=1.0)
            rs32 = stp.tile([128, SW], f32, tag="rs32", name="rs32")
            nc.vector.reciprocal(out=rs32, in_=sd)
            rsmu = stp.tile([128, 2 * SW], f16, tag=f"rsmu{sc}",
                            name=f"rsmu{sc}")
            nc.vector.tensor_scalar_mul(out=rsmu[:, 0:SW], in0=rs32,
                                        scalar1=1.0)
            nc.vector.scalar_tensor_tensor(out=rsmu[:, SW:2 * SW], in0=spk,
                                           scalar=1.0 / C, in1=rs32,
                                           op0=OP.mult, op1=OP.mult)
            rsmu_l.append(rsmu)
            nc.gpsimd.dma_start(
                out=scr_d[sc:sc + 1, :].rearrange("one (p w) -> one p w",
                                                  p=128),
                in_=rsmu)

        for sc in range(NSC):
            halves = []
            row = scr_d[sc:sc + 1, :]
            for rh in range(2):
                rb = rbp.tile([128, SC], f16, tag=f"rb{rh}", name="rb")
                hw_dma(
                    out=rb,
                    in_=bass.AP(tensor=row.tensor,
                                offset=row.offset + rh * SC,
                                ap=[[0, 128], [1, SC]]))
                halves.append(rb)
            rb_l[sc] = halves

        # per-half state
        qm_r = [None] * NHALF
        xa_r = [None] * NHALF
        prev_hi = {0: -1, 1: -1}

        def windows_dma(h, s, ct, dma):
            # xa window for conv chunk s of half h, channel tile ct
            st = h * LH + s * CH
            dma(
                out=xa_r[h][ct * 64 + s:ct * 64 + s + 49:16, :]
                .rearrange("g (cc w) -> g cc w", cc=GC),
                in_=xa_full[ct][:, st:st + WIN])

        def flush_windows(h, s_hi):
            # inline: only SWDGE windows (HWDGE-ring ones would stall the
            # issuing engine's SEQ between gelus); HWDGE half is deferred
            # to emit_ph2b once the half's gelus are all emitted.
            s_hi = min(s_hi, NCH_H - 1)
            for s in range(prev_hi[h] + 1, s_hi + 1):
                for ct in range(NCT):
                    if (s + ct) % 2 == 1:
                        windows_dma(h, s, ct, nc.gpsimd.dma_start)
            prev_hi[h] = max(prev_hi[h], s_hi)

        def flush_windows_hw(h):
            for s in range(NCH_H):
                for ct in range(NCT):
                    if (s + ct) % 2 == 0:
                        windows_dma(h, s, ct, hw_dma)

        # ============= phase 2: apply + gelu + om (per half) =============
        APW = 1024  # apply op width
        SC_PER_H = LH // SC  # 2

        def emit_apply(sc, lo_off, width):
            # LN apply + gelu for x columns [sc*SC+lo_off, +width)
            rh = (lo_off // SW) // 64
            rb_v = rb_l[sc][rh][:].rearrange("q (p half w) -> q p half w",
                                             p=64, half=2)
            rs_sl = rb_v[:, :, 0, :]
            mu_sl = rb_v[:, :, 1, :]
            lo = sc * SC + lo_off
            pw = width // SW
            p0 = (lo_off // SW) % 64
            for ct in range(NCT):
                xs = x16[ct][:, lo:lo + width].rearrange(
                    "p (a w) -> p a w", a=pw)
                v = vup.tile([128, APW], f16, tag=f"v{ct}", name=f"v{ct}")
                vv = v[:, 0:width].rearrange("p (a w) -> p a w", a=pw)
                nc.vector.tensor_tensor(
                    out=vv, in0=xs, in1=rs_sl[:, p0:p0 + pw, :], op=OP.mult)
                nc.vector.tensor_tensor(
                    out=vv, in0=vv, in1=mu_sl[:, p0:p0 + pw, :],
                    op=OP.subtract)
                nc.scalar.activation(
                    out=xa_full[ct][:, HALO + lo:HALO + lo + width],
                    in_=v[:, 0:width], func=GELU, bias=lnb_c[ct],
                    scale=lnw_c[ct])

        def emit_ph2a(h):
            xa_r[h] = xrp.tile([128, GC * WIN], f16, tag="xa_r",
                               name=f"xa_r{h}")
            for sc_h in range(SC_PER_H):
                sc = h * SC_PER_H + sc_h
                for a2 in range(SC // APW):
                    emit_apply(sc, a2 * APW, APW)
                    cov = sc_h * SC + (a2 + 1) * APW
                    flush_windows(h, (cov - CH - HALO) // CH)
            if h == 0:
                # pre-compute h1's first 512 cols so h0's last window (and
                # thus h0's conv) doesn't wait for h1's front-end. h1
                # recomputes the same values later (idempotent).
                emit_apply(SC_PER_H, 0, 512)
                flush_windows(h, NCH_H - 1)

        def emit_ph2b(h):
            qm_r[h] = qmp.tile([128, 2 * K * LQ], f16, tag="qm_r",
                               name=f"qm_r{h}")
            if h == NHALF - 1:
                flush_windows(h, NCH_H - 1)
            flush_windows_hw(h)
            for gp in range(LH // 512):
                lo5 = h * LH + gp * 512
                om_ps = psum_o.tile([48, 512], f32, tag="om", name="om")
                for ct in range(NCT):
                    nc.tensor.matmul(
                        om_ps, omwT[ct],
                        xa_full[ct][:, HALO + lo5:HALO + lo5 + 512],
                        start=(ct == 0), stop=(ct == NCT - 1))
                om_st = omp.tile([48, 512], f16, tag="om_st", name="om_st")
                nc.scalar.activation(out=om_st, in_=om_ps,
                                     func=AF.Identity, bias=bias48,
                                     scale=1.0)
                for s2 in range(2):
                    s = gp * 2 + s2  # conv chunk within half
                    hw_dma(
                        out=qm_r[h][s:128:16, :]
                        .rearrange("g (o f) -> g o f", o=6),
                        in_=om_st[:, s2 * 256:(s2 + 1) * 256])

        # ================= phase 3: banded conv per half =================
        def emit_ph3(h):
            q_ap = qm_r[h][:, 0:K * LQ]
            m_ap = qm_r[h][:, K * LQ:2 * K * LQ]
            hm_all = cfp.tile([128, ND * K * LQ], f16, tag="hm",
                              name=f"hm{h}")
            r2t = cfp.tile([128, K * LQ], f16, tag="r2t", name="r2t")
            for i in range(ND):
                d = D_LO + i
                sl = hm_all[:, i * K * LQ:(i + 1) * K * LQ]
                nc.vector.tensor_scalar_add(out=sl, in0=q_ap,
                                            scalar1=float(1 - d))
                nc.vector.tensor_scalar(out=r2t, in0=q_ap,
                                        scalar1=float(d + 1), scalar2=-1.0,
                                        op0=OP.subtract, op1=OP.mult)
                nc.vector.tensor_tensor(out=sl, in0=sl, in1=r2t, op=OP.min)
                nc.vector.tensor_scalar_max(out=sl, in0=sl, scalar1=0.0)
            m_rep = bass.AP(tensor=m_ap.tensor, offset=m_ap.offset,
                            ap=[m_ap.ap[0], [0, ND], [1, K * LQ]])
            nc.vector.tensor_tensor(
                out=hm_all[:].rearrange("p (d f) -> p d f", d=ND),
                in0=hm_all[:].rearrange("p (d f) -> p d f", d=ND),
                in1=m_rep, op=OP.mult)
            cd_all = cfp.tile([128, ND * LQ], f16, tag="cd", name=f"cd{h}")
            hm_v = hm_all[:].rearrange("p (d k f) -> p d k f", d=ND, k=K)
            cd_v = cd_all[:].rearrange("p (d f) -> p d f", d=ND)
            nc.vector.tensor_tensor(out=cd_v, in0=hm_v[:, :, 0, :],
                                    in1=hm_v[:, :, 1, :], op=OP.add)
            nc.vector.tensor_tensor(out=cd_v, in0=cd_v,
                                    in1=hm_v[:, :, 2, :], op=OP.add)

            xa_r_v = xa_r[h][:].rearrange("p (c wn) -> p c wn", c=GC)
            out_t = out_d[:]
            ost_all = outp.tile([128, GC * LQ], f16, tag="ost_all",
                                name=f"ost_all{h}")
            for cp in range(NCP):
                tmps = []
                for i in range(ND):
                    d = D_LO + i
                    tmp = tmpp.tile([128, CPG * LQ], f16, tag=f"tmp{i}",
                                    name=f"tmp{i}")
                    cdi = cd_all[:, i * LQ:(i + 1) * LQ]
                    cb = bass.AP(tensor=cdi.tensor, offset=cdi.offset,
                                 ap=[cdi.ap[0], [0, CPG], [1, LQ]])
                    nc.vector.tensor_tensor(
                        out=tmp[:].rearrange("p (c f) -> p c f", c=CPG),
                        in0=xa_r_v[:, cp * CPG:(cp + 1) * CPG,
                                   HALO + d:HALO + d + LQ],
                        in1=cb, op=OP.mult)
                    tmps.append(tmp)
                o0 = cp * CPG * LQ
                if cp in DSUM_DVE_CP:
                    nc.vector.tensor_tensor(out=tmps[0], in0=tmps[0],
                                            in1=tmps[1], op=OP.add)
                    nc.vector.tensor_tensor(out=tmps[2], in0=tmps[2],
                                            in1=tmps[3], op=OP.add)
                    nc.vector.tensor_tensor(out=tmps[0], in0=tmps[0],
                                            in1=tmps[2], op=OP.add)
                    nc.vector.tensor_tensor(
                        out=ost_all[:, o0:o0 + CPG * LQ], in0=tmps[0],
                        in1=tmps[4], op=OP.add)
                else:
                    for jp in range(CPG // 2):
                        acc = psum_a.tile([128, 2 * LQ], f32, tag="acc",
                                          name="acc")
                        for jj in range(2):
                            j = jp * 2 + jj
                            for n_i in range(ND):
                                nc.tensor.matmul(
                                    acc[:, jj * LQ:(jj + 1) * LQ], id16,
                                    tmps[n_i][:, j * LQ:(j + 1) * LQ],
                                    start=(n_i == 0), stop=(n_i == ND - 1))
                        oj = o0 + jp * 2 * LQ
                        if (cp, jp) in OST_POOL_JP:
                            nc.gpsimd.tensor_copy(
                                out=ost_all[:, oj:oj + 2 * LQ], in_=acc)
                        else:
                            nc.scalar.copy(
                                out=ost_all[:, oj:oj + 2 * LQ], in_=acc)
            for g in range(G):
                hw_dma(
                    out=bass.AP(
                        tensor=out_t.tensor,
                        offset=out_t.offset + g * GC * L + h * LH,
                        ap=[[CH, NCH_H], [L, GC], [1, LQ]]),
                    in_=ost_all[g * 16:g * 16 + 16, :]
                    .rearrange("s (c f) -> s c f", c=GC))

        # driver: front-ends first (conv overlaps them via per-engine
        # in-order streams)
        emit_ph2a(0)
        emit_ph2b(0)
        emit_ph2a(1)
        emit_ph2b(1)
        emit_ph3(0)
        emit_ph3(1)

    return nc


def _prep_params(ln_w, ln_b, om_w, om_b):
    cst32 = np.zeros((128, 6), np.float32)
    cst32[:, 0] = ln_w[0:128]
    cst32[:, 1] = ln_w[128:256]
    cst32[:, 2] = ln_b[0:128]
    cst32[:, 3] = ln_b[128:256]
    cst32[:, 4] = 1.0
    # bias48 in (g, qm, k) order with conv grid fold for qm=0 (offsets)
    for g in range(G):
        for k in range(K):
            cst32[g * 6 + k, 5] = om_b[g * K + k] + (k - 1.0)
            cst32[g * 6 + 3 + k, 5] = om_b[G * K + g * K + k]
    cst16 = np.zeros((128, 226), np.float16)
    omwT = om_w.T.astype(np.float16)          # [C, 48] original row order
    perm = np.zeros(48, np.int64)
    for g in range(G):
        for k in range(K):
            perm[g * 6 + k] = g * K + k
            perm[g * 6 + 3 + k] = G * K + g * K + k
    omwTp = omwT[:, perm]                      # columns in (g,qm,k) order
    cst16[:, 0:48] = omwTp[0:128]
    cst16[:, 48:96] = omwTp[128:256]
    cst16[:, 96:224] = np.eye(128, dtype=np.float16)
    cst16[:, 224] = 1.0
    return {"cst32": cst32, "cst16": cst16}


def kernel(x, ln_w, ln_b, om_w, om_b):
    _install_patch()
    from concourse.bass_utils import run_bass_kernel_spmd

    if "nc" not in _cache:
        _cache["nc"] = _build_module()
    nc = _cache["nc"]

    x = np.ascontiguousarray(np.asarray(x, dtype=np.float32))
    params = _prep_params(np.asarray(ln_w, np.float32),
                          np.asarray(ln_b, np.float32),
                          np.asarray(om_w, np.float32),
                          np.asarray(om_b, np.float32))
    in_maps = [dict(params, x=x[n]) for n in range(N)]
    res = run_bass_kernel_spmd(nc, in_maps, core_ids=list(range(N)))
    return np.stack([res.results[n]["out"] for n in range(N)],
                    axis=0).astype(np.float32)


def _prep_inputs(inputs):
    x = np.ascontiguousarray(np.asarray(inputs["x"], dtype=np.float32))
    params = _prep_params(np.asarray(inputs["ln_w"], np.float32),
                          np.asarray(inputs["ln_b"], np.float32),
                          np.asarray(inputs["om_w"], np.float32),
                          np.asarray(inputs["om_b"], np.float32))
    return [dict(params, x=x[n]) for n in range(N)]


def run_traced(inputs):
    _install_patch()
    from concourse.bass_utils import run_bass_kernel_spmd
    if "nc" not in _cache:
        _cache["nc"] = _build_module()
    return run_bass_kernel_spmd(_cache["nc"], _prep_inputs(inputs),
                                core_ids=list(range(N)), trace=True)


# revision 45
# speedup vs baseline: 1.0573x; 1.0075x over previous
"""DCNv4-1D fused Trainium2 kernel v2. Data-parallel over batch N, 8 cores.

Per core (one sample), redesigned against the TimelineSim cost model:
  - x loaded via SWDGE cast-DMA f32->f16 (cost model charges OUT bytes).
  - LN stats: f16 ones-matmuls on PE into PSUM [33,512] (sum row 0, sumsq
    row 32); ACT copies to an f16 row; tiny DMAs repack to [128,w]; DVE/ACT
    finish rs/murs; DRAM round-trip broadcasts rs|murs to [128, 2*SC] f16.
    All Sqrt before any Gelu (2 ACT table loads total).
  - Apply: v = x16*rs_b, u = v - murs_b (DVE f16 TT, 2x mode), then
    xa = gelu(lnw*u + lnb) on ACT into xa_full with zeroed halo guards.
  - om = om_wT' @ xa on PE (f16), ACT adds bias(+grid fold), scatter DMAs
    land rows in qm_r [(g,s16), (o,f)] layout.
  - Banded conv (5 diagonals d=-2..2): coeff on DVE (tensor_scalar 4x via
    abs_max trick + TT mask/ksum); products coeff*xa on DVE f16 TT;
    d-sum mostly via PE id16 matmuls into PSUM + ACT copy-out, partly as
    DVE TT adds writing f16 directly (knob).
  - L split in 2 halves: half-0 conv overlaps half-1 front-end.
"""

import json

import numpy as np

N, C, L = 8, 256, 8192
G, K, GC = 8, 3, 32
LN_EPS = 1e-6
NCT = 2
NHALF = 2
LH = L // NHALF          # 4096
SC = 2048                # stats superchunk
NSC = L // SC            # 4 (2 per half)
SW = SC // 128           # 16 stats cols per partition per sc
CH = 256                 # conv chunk (16 per half -> partitions (g, s))
NCH_H = LH // CH         # 16
LQ = CH
HALO = 4
WIN = CH + 2 * HALO      # 264
D_LO, D_HI = -2, 2
ND = D_HI - D_LO + 1     # 5
CPG = 4                  # channels per product op
NCP = GC // CPG          # 4 product blocks per half

# ---- tuning knobs ----
DSUM_DVE_CP = ()         # cp blocks whose d-sum runs on DVE (rest PE+ACT)
OST_POOL_JP = ()         # (cp, jp) pairs whose PSUM->f16 copy runs on Pool
SCAT_SWDGE = True        # half the qm scatters on SWDGE (Pool) vs HWDGE

_cache = {}
_DBG_PSUM_INIT = False
_DBG_RELU = False


# --- BIR post-pass: this walrus build rejects >1 sync wait per instruction;
# split extras onto same-engine NoOps inserted just before the owner. ---
def _split_multi_waits(bir_json: bytes, max_waits: int = 1) -> bytes:
    j = json.loads(bir_json)
    n = [0]

    def fresh():
        n[0] += 1
        return f"I-wsplit-{n[0]}"

    for fn in j.get("functions", []):
        for bb in fn.get("basicblocks", []) or fn.get("blocks", []) or []:
            out = []
            for inst in bb.get("instructions", []):
                si = inst.get("sync_info")
                waits = (si or {}).get("on_wait") or []
                if len(waits) > max_waits:
                    for w in waits[:-max_waits]:
                        out.append({
                            "debug": inst.get("debug", 0),
                            "engine": inst["engine"],
                            "ins": [], "outs": [],
                            "name": fresh(),
                            "opcode": "NoOp",
                            "sync_info": {"on_update": [], "on_wait": [w]},
                        })
                    si["on_wait"] = waits[-max_waits:]
                out.append(inst)
            bb["instructions"] = out
    return json.dumps(j).encode()


def _install_patch():
    import os

    # The elide-DMA sem optimization assumes per-ring FIFO but DMAHW lanes
    # are shared across the SP/ACT rings; with our cross-ring interleave it
    # miscounts and consumers fire early. Disable it.
    os.environ["BACC_ELIDE_DMA_OPT_LIMIT"] = "0"
    import concourse.tile_sem_assignment as _tsa
    _tsa._opt_limit = 0

    import concourse.bass2jax as bass2jax
    import concourse.bass_utils as bass_utils

    if getattr(bass2jax.compile_bir_kernel, "_wsplit", False):
        return
    orig = bass_utils.compile_bir_kernel

    def patched(bir_json, tmpdir, neff_name="file.neff"):
        return orig(_split_multi_waits(bir_json), tmpdir, neff_name=neff_name)

    patched._wsplit = True
    bass_utils.compile_bir_kernel = patched
    bass2jax.compile_bir_kernel = patched


def _build_module():
    import contextlib

    import concourse.bass as bass
    import concourse.tile as tile
    from concourse import mybir

    f32 = mybir.dt.float32
    f16 = mybir.dt.float16
    AF = mybir.ActivationFunctionType
    GELU = AF.Relu if _DBG_RELU else AF.Gelu
    OP = mybir.AluOpType

    nc = bass.Bass()

    x_d = nc.dram_tensor("x", [C, L], f32, kind="ExternalInput")
    cst32_d = nc.dram_tensor("cst32", [128, 6], f32, kind="ExternalInput")
    cst16_d = nc.dram_tensor("cst16", [128, 226], f16, kind="ExternalInput")
    scr_d = nc.dram_tensor("scr", [NSC, 2 * SC], f16, kind="Internal")
    out_d = nc.dram_tensor("out", [C, L], f16, kind="ExternalOutput")

    with tile.TileContext(nc) as tc, contextlib.ExitStack() as ctx:
        const = ctx.enter_context(tc.tile_pool(name="const", bufs=1))
        persist = ctx.enter_context(tc.tile_pool(name="persist", bufs=1))
        xqp = ctx.enter_context(tc.tile_pool(name="xq", bufs=1))
        srp = ctx.enter_context(tc.tile_pool(name="srp", bufs=2))
        stp = ctx.enter_context(tc.tile_pool(name="stp", bufs=2))
        rbp = ctx.enter_context(tc.tile_pool(name="rbp", bufs=3))
        vup = ctx.enter_context(tc.tile_pool(name="vup", bufs=2))
        omp = ctx.enter_context(tc.tile_pool(name="omp", bufs=2))
        xrp = ctx.enter_context(tc.tile_pool(name="xrp", bufs=2))
        qmp = ctx.enter_context(tc.tile_pool(name="qmp", bufs=2))
        cfp = ctx.enter_context(tc.tile_pool(name="cfp", bufs=1))
        tmpp = ctx.enter_context(tc.tile_pool(name="tmpp", bufs=2))
        outp = ctx.enter_context(tc.tile_pool(name="outp", bufs=1))
        psum = ctx.enter_context(tc.tile_pool(name="psum", bufs=2,
                                              space="PSUM"))
        psum_o = ctx.enter_context(tc.tile_pool(name="psum_o", bufs=2,
                                                space="PSUM"))
        psum_a = ctx.enter_context(tc.tile_pool(name="psum_a", bufs=4,
                                                space="PSUM"))

        # HWDGE lane purity: DMAHW lanes are assigned round-robin over ALL
        # hwdge DMAs; threshold waits are only sound if each lane is fed by
        # one FIFO ring. Strict SP/ACT alternation keeps even lanes SP-only
        # and odd lanes ACT-only (NUM_HWDGE_SEMS == 8).
        _hwflip = [0]

        def hw_dma(out, in_):
            eng = nc.sync if _hwflip[0] % 2 == 0 else nc.scalar
            _hwflip[0] += 1
            eng.dma_start(out=out, in_=in_)

        # ---------------- constants ----------------
        cst32 = const.tile([128, 6], f32, tag="cst32", name="cst32")
        hw_dma(out=cst32, in_=cst32_d[:])
        cst16 = const.tile([128, 226], f16, tag="cst16", name="cst16")
        hw_dma(out=cst16, in_=cst16_d[:])
        lnw_c = [cst32[:, ct:ct + 1] for ct in range(NCT)]
        lnb_c = [cst32[:, 2 + ct:3 + ct] for ct in range(NCT)]
        bias48 = cst32[0:48, 5:6]
        omwT = [cst16[:, ct * 48:(ct + 1) * 48] for ct in range(NCT)]
        id16 = cst16[:, 96:224]
        onesc16 = cst16[:, 224:225]
        eps_c = const.tile([128, 1], f32, tag="eps", name="eps")
        nc.vector.memset(eps_c, LN_EPS)

        # ---------------- persistent tensors ----------------
        x16 = [persist.tile([128, L], f16, tag=f"x16_{ct}",
                            name=f"x16_{ct}") for ct in range(NCT)]
        xa_full = [persist.tile([128, 2 * HALO + L], f16, tag=f"xaf{ct}",
                                name=f"xaf{ct}") for ct in range(NCT)]
        for ct in range(NCT):
            nc.vector.memset(xa_full[ct][:, 0:HALO], 0.0)
            nc.vector.memset(xa_full[ct][:, HALO + L:2 * HALO + L], 0.0)

        # x cast loads (SWDGE), chunked per superchunk so stats start early
        for sc in range(NSC):
            for ct in range(NCT):
                nc.gpsimd.dma_start(
                    out=x16[ct][:, sc * SC:(sc + 1) * SC],
                    in_=x_d[ct * 128:(ct + 1) * 128, sc * SC:(sc + 1) * SC])

        rb_l = [None] * NSC

        # ============ phase 1: LN stats, staged across superchunks ============
        # (same-stage DMAs are adjacent in each queue so the four superchunk
        # round-trips pipeline instead of serializing end-to-end)
        spk_l, qpk_l, rsmu_l = [], [], []
        for sc in range(NSC):
            lo = sc * SC
            srow = srp.tile([33, SC], f16, tag="srow", name="srow")
            xq = [None] * NCT
            for ct in range(NCT):
                xq[ct] = xqp.tile([128, SC], f16, tag=f"xq{ct}",
                                  name=f"xq{ct}")
                nc.vector.tensor_tensor(out=xq[ct],
                                        in0=x16[ct][:, lo:lo + SC],
                                        in1=x16[ct][:, lo:lo + SC],
                                        op=OP.mult)
            for c in range(SC // 512):
                cf = c * 512
                st_ps = psum.tile([33, 512], f32, tag="st", name="st")
                if _DBG_PSUM_INIT:
                    nc.vector.memset(st_ps[1:32, :], 0.0)
                for ct in range(NCT):
                    nc.tensor.matmul(st_ps[0:1, :], onesc16,
                                     x16[ct][:, lo + cf:lo + cf + 512],
                                     start=(ct == 0), stop=(ct == NCT - 1))
                for ct in range(NCT):
                    nc.tensor.matmul(st_ps[32:33, :], onesc16,
                                     xq[ct][:, cf:cf + 512],
                                     start=(ct == 0), stop=(ct == NCT - 1))
                nc.scalar.copy(out=srow[:, cf:cf + 512], in_=st_ps)

            spk = stp.tile([128, SW], f16, tag=f"spk{sc}", name=f"spk{sc}")
            hw_dma(
                out=spk,
                in_=srow[0:1, :].rearrange("one (p w) -> one p w", p=128))
            qpk = stp.tile([128, SW], f16, tag=f"qpk{sc}", name=f"qpk{sc}")
            hw_dma(
                out=qpk,
                in_=srow[32:33, :].rearrange("one (p w) -> one p w", p=128))
            spk_l.append(spk)
            qpk_l.append(qpk)

        for sc in range(NSC):
            spk, qpk = spk_l[sc], qpk_l[sc]
            mu_t = stp.tile([128, SW], f32, tag="mu_t", name="mu_t")
            nc.vector.tensor_scalar_mul(out=mu_t, in0=spk, scalar1=1.0 / C)
            musq = stp.tile([128, SW], f32, tag="musq", name="musq")
            nc.vector.tensor_tensor(out=musq, in0=mu_t, in1=mu_t,
                                    op=OP.mult)
            varq = stp.tile([128, SW], f32, tag="varq", name="varq")
            nc.vector.scalar_tensor_tensor(out=varq, in0=qpk, scalar=1.0 / C,
                                           in1=musq, op0=OP.mult,
                                           op1=OP.subtract)
            sd = stp.tile([128, SW], f32, tag="sd", name="sd")
            nc.scalar.activation(out=sd, in_=varq, func=AF.Sqrt,
                                 bias=eps_c, scale=1.0)
            rs32 = stp.tile([128, SW], f32, tag="rs32", name="rs32")
            nc.vector.reciprocal(out=rs32, in_=sd)
            rsmu = stp.tile([128, 2 * SW], f16, tag=f"rsmu{sc}",
                            name=f"rsmu{sc}")
            nc.vector.tensor_scalar_mul(out=rsmu[:, 0:SW], in0=rs32,
                                        scalar1=1.0)
            nc.vector.scalar_tensor_tensor(out=rsmu[:, SW:2 * SW], in0=spk,
                                           scalar=1.0 / C, in1=rs32,
                                           op0=OP.mult, op1=OP.mult)
            rsmu_l.append(rsmu)
            nc.gpsimd.dma_start(
                out=scr_d[sc:sc + 1, :].rearrange("one (p w) -> one p w",
                                                  p=128),
                in_=rsmu)

        for sc in range(NSC):
            halves = []
            row = scr_d[sc:sc + 1, :]
            for rh in range(2):
                rb = rbp.tile([128, SC], f16, tag=f"rb{rh}", name="rb")
                hw_dma(
                    out=rb,
                    in_=bass.AP(tensor=row.tensor,
                                offset=row.offset + rh * SC,
                                ap=[[0, 128], [1, SC]]))
                halves.append(rb)
            rb_l[sc] = halves

        # per-half state
        qm_r = [None] * NHALF
        xa_r = [None] * NHALF
        prev_hi = {0: -1, 1: -1}

        def windows_dma(h, s, ct, dma):
            # xa window for conv chunk s of half h, channel tile ct
            st = h * LH + s * CH
            dma(
                out=xa_r[h][ct * 64 + s:ct * 64 + s + 49:16, :]
                .rearrange("g (cc w) -> g cc w", cc=GC),
                in_=xa_full[ct][:, st:st + WIN])

        def flush_windows(h, s_hi):
            # inline: only SWDGE windows (HWDGE-ring ones would stall the
            # issuing engine's SEQ between gelus); HWDGE half is deferred
            # to emit_ph2b once the half's gelus are all emitted.
            s_hi = min(s_hi, NCH_H - 1)
            for s in range(prev_hi[h] + 1, s_hi + 1):
                for ct in range(NCT):
                    if (s + ct) % 2 == 1:
                        windows_dma(h, s, ct, nc.gpsimd.dma_start)
            prev_hi[h] = max(prev_hi[h], s_hi)

        def flush_windows_hw(h):
            for s in range(NCH_H):
                for ct in range(NCT):
                    if (s + ct) % 2 == 0:
                        windows_dma(h, s, ct, hw_dma)

        # ============= phase 2: apply + gelu + om (per half) =============
        APW = 1024  # apply op width
        SC_PER_H = LH // SC  # 2

        def emit_apply(sc, lo_off, width):
            # LN apply + gelu for x columns [sc*SC+lo_off, +width)
            rh = (lo_off // SW) // 64
            rb_v = rb_l[sc][rh][:].rearrange("q (p half w) -> q p half w",
                                             p=64, half=2)
            rs_sl = rb_v[:, :, 0, :]
            mu_sl = rb_v[:, :, 1, :]
            lo = sc * SC + lo_off
            pw = width // SW
            p0 = (lo_off // SW) % 64
            for ct in range(NCT):
                xs = x16[ct][:, lo:lo + width].rearrange(
                    "p (a w) -> p a w", a=pw)
                v = vup.tile([128, APW], f16, tag=f"v{ct}", name=f"v{ct}")
                vv = v[:, 0:width].rearrange("p (a w) -> p a w", a=pw)
                nc.vector.tensor_tensor(
                    out=vv, in0=xs, in1=rs_sl[:, p0:p0 + pw, :], op=OP.mult)
                nc.vector.tensor_tensor(
                    out=vv, in0=vv, in1=mu_sl[:, p0:p0 + pw, :],
                    op=OP.subtract)
                nc.scalar.activation(
                    out=xa_full[ct][:, HALO + lo:HALO + lo + width],
                    in_=v[:, 0:width], func=GELU, bias=lnb_c[ct],
                    scale=lnw_c[ct])

        def emit_ph2a(h):
            xa_r[h] = xrp.tile([128, GC * WIN], f16, tag="xa_r",
                               name=f"xa_r{h}")
            for sc_h in range(SC_PER_H):
                sc = h * SC_PER_H + sc_h
                for a2 in range(SC // APW):
                    emit_apply(sc, a2 * APW, APW)
                    cov = sc_h * SC + (a2 + 1) * APW
                    flush_windows(h, (cov - CH - HALO) // CH)
            if h == 0:
                # pre-compute h1's first 512 cols so h0's last window (and
                # thus h0's conv) doesn't wait for h1's front-end. h1
                # recomputes the same values later (idempotent).
                emit_apply(SC_PER_H, 0, 512)
                flush_windows(h, NCH_H - 1)

        def emit_ph2b(h):
            qm_r[h] = qmp.tile([128, 2 * K * LQ], f16, tag="qm_r",
                               name=f"qm_r{h}")
            if h == NHALF - 1:
                flush_windows(h, NCH_H - 1)
            flush_windows_hw(h)
            for gp in range(LH // 512):
                lo5 = h * LH + gp * 512
                om_ps = psum_o.tile([48, 512], f32, tag="om", name="om")
                for ct in range(NCT):
                    nc.tensor.matmul(
                        om_ps, omwT[ct],
                        xa_full[ct][:, HALO + lo5:HALO + lo5 + 512],
                        start=(ct == 0), stop=(ct == NCT - 1))
                om_st = omp.tile([48, 512], f16, tag="om_st", name="om_st")
                nc.scalar.activation(out=om_st, in_=om_ps,
                                     func=AF.Identity, bias=bias48,
                                     scale=1.0)
                for s2 in range(2):
                    s = gp * 2 + s2  # conv chunk within half
                    hw_dma(
                        out=qm_r[h][s:128:16, :]
                        .rearrange("g (o f) -> g o f", o=6),
                        in_=om_st[:, s2 * 256:(s2 + 1) * 256])

        # ================= phase 3: banded conv per half =================
        def emit_ph3(h):
            q_ap = qm_r[h][:, 0:K * LQ]
            m_ap = qm_r[h][:, K * LQ:2 * K * LQ]
            hm_all = cfp.tile([128, ND * K * LQ], f16, tag="hm",
                              name=f"hm{h}")
            r2t = cfp.tile([128, K * LQ], f16, tag="r2t", name="r2t")
            for i in range(ND):
                d = D_LO + i
                sl = hm_all[:, i * K * LQ:(i + 1) * K * LQ]
                nc.vector.tensor_scalar_add(out=sl, in0=q_ap,
                                            scalar1=float(1 - d))
                nc.vector.tensor_scalar(out=r2t, in0=q_ap,
                                        scalar1=float(d + 1), scalar2=-1.0,
                                        op0=OP.subtract, op1=OP.mult)
                nc.vector.tensor_tensor(out=sl, in0=sl, in1=r2t, op=OP.min)
                nc.vector.tensor_scalar_max(out=sl, in0=sl, scalar1=0.0)
            m_rep = bass.AP(tensor=m_ap.tensor, offset=m_ap.offset,
                            ap=[m_ap.ap[0], [0, ND], [1, K * LQ]])
            nc.vector.tensor_tensor(
                out=hm_all[:].rearrange("p (d f) -> p d f", d=ND),
                in0=hm_all[:].rearrange("p (d f) -> p d f", d=ND),
                in1=m_rep, op=OP.mult)
            cd_all = cfp.tile([128, ND * LQ], f16, tag="cd", name=f"cd{h}")
            hm_v = hm_all[:].rearrange("p (d k f) -> p d k f", d=ND, k=K)
            cd_v = cd_all[:].rearrange("p (d f) -> p d f", d=ND)
            nc.vector.tensor_tensor(out=cd_v, in0=hm_v[:, :, 0, :],
                                    in1=hm_v[:, :, 1, :], op=OP.add)
            nc.vector.tensor_tensor(out=cd_v, in0=cd_v,
                                    in1=hm_v[:, :, 2, :], op=OP.add)

            xa_r_v = xa_r[h][:].rearrange("p (c wn) -> p c wn", c=GC)
            out_t = out_d[:]
            ost_all = outp.tile([128, GC * LQ], f16, tag="ost_all",
                                name=f"ost_all{h}")
            for cp in range(NCP):
                tmps = []
                for i in range(ND):
                    d = D_LO + i
                    tmp = tmpp.tile([128, CPG * LQ], f16, tag=f"tmp{i}",
                                    name=f"tmp{i}")
                    cdi = cd_all[:, i * LQ:(i + 1) * LQ]
                    cb = bass.AP(tensor=cdi.tensor, offset=cdi.offset,
                                 ap=[cdi.ap[0], [0, CPG], [1, LQ]])
                    nc.vector.tensor_tensor(
                        out=tmp[:].rearrange("p (c f) -> p c f", c=CPG),
                        in0=xa_r_v[:, cp * CPG:(cp + 1) * CPG,
                                   HALO + d:HALO + d + LQ],
                        in1=cb, op=OP.mult)
                    tmps.append(tmp)
                o0 = cp * CPG * LQ
                if cp in DSUM_DVE_CP:
                    nc.vector.tensor_tensor(out=tmps[0], in0=tmps[0],
                                            in1=tmps[1], op=OP.add)
                    nc.vector.tensor_tensor(out=tmps[2], in0=tmps[2],
                                            in1=tmps[3], op=OP.add)
                    nc.vector.tensor_tensor(out=tmps[0], in0=tmps[0],
                                            in1=tmps[2], op=OP.add)
                    nc.vector.tensor_tensor(
                        out=ost_all[:, o0:o0 + CPG * LQ], in0=tmps[0],
                        in1=tmps[4], op=OP.add)
                else:
                    for jp in range(CPG // 2):
                        acc = psum_a.tile([128, 2 * LQ], f32, tag="acc",
                                          name="acc")
                        for jj in range(2):
                            j = jp * 2 + jj
                            for n_i in range(ND):
                                nc.tensor.matmul(
                                    acc[:, jj * LQ:(jj + 1) * LQ], id16,
                                    tmps[n_i][:, j * LQ:(j + 1) * LQ],
                                    start=(n_i == 0), stop=(n_i == ND - 1))
                        oj = o0 + jp * 2 * LQ
                        if (cp, jp) in OST_POOL_JP:
                            nc.gpsimd.tensor_copy(
                                out=ost_all[:, oj:oj + 2 * LQ], in_=acc)
                        else:
                            nc.scalar.copy(
                                out=ost_all[:, oj:oj + 2 * LQ], in_=acc)
            for g in range(G):
                hw_dma(
                    out=bass.AP(
                        tensor=out_t.tensor,
                        offset=out_t.offset + g * GC * L + h * LH,
                        ap=[[CH, NCH_H], [L, GC], [1, LQ]]),
                    in_=ost_all[g * 16:g * 16 + 16, :]
                    .rearrange("s (c f) -> s c f", c=GC))

        # driver: front-ends first (conv overlaps them via per-engine
        # in-order streams)
        emit_ph2a(0)
        emit_ph2a(1)
        emit_ph2b(0)
        emit_ph2b(1)
        emit_ph3(0)
        emit_ph3(1)

    return nc


def _prep_params(ln_w, ln_b, om_w, om_b):
    cst32 = np.zeros((128, 6), np.float32)
    cst32[:, 0] = ln_w[0:128]
    cst32[:, 1] = ln_w[128:256]
    cst32[:, 2] = ln_b[0:128]
    cst32[:, 3] = ln_b[128:256]
    cst32[:, 4] = 1.0
    # bias48 in (g, qm, k) order with conv grid fold for qm=0 (offsets)
    for g in range(G):
        for k in range(K):
            cst32[g * 6 + k, 5] = om_b[g * K + k] + (k - 1.0)
            cst32[g * 6 + 3 + k, 5] = om_b[G * K + g * K + k]
    cst16 = np.zeros((128, 226), np.float16)
    omwT = om_w.T.astype(np.float16)          # [C, 48] original row order
    perm = np.zeros(48, np.int64)
    for g in range(G):
        for k in range(K):
            perm[g * 6 + k] = g * K + k
            perm[g * 6 + 3 + k] = G * K + g * K + k
    omwTp = omwT[:, perm]                      # columns in (g,qm,k) order
    cst16[:, 0:48] = omwTp[0:128]
    cst16[:, 48:96] = omwTp[128:256]
    cst16[:, 96:224] = np.eye(128, dtype=np.float16)
    cst16[:, 224] = 1.0
    return {"cst32": cst32, "cst16": cst16}


def kernel(x, ln_w, ln_b, om_w, om_b):
    _install_patch()
    from concourse.bass_utils import run_bass_kernel_spmd

    if "nc" not in _cache:
        _cache["nc"] = _build_module()
    nc = _cache["nc"]

    x = np.ascontiguousarray(np.asarray(x, dtype=np.float32))
    params = _prep_params(np.asarray(ln_w, np.float32),
                          np.asarray(ln_b, np.float32),
                          np.asarray(om_w, np.float32),
                          np.asarray(om_b, np.float32))
    in_maps = [dict(params, x=x[n]) for n in range(N)]
    res = run_bass_kernel_spmd(nc, in_maps, core_ids=list(range(N)))
    return np.stack([res.results[n]["out"] for n in range(N)],
                    axis=0).astype(np.float32)


def _prep_inputs(inputs):
    x = np.ascontiguousarray(np.asarray(inputs["x"], dtype=np.float32))
    params = _prep_params(np.asarray(inputs["ln_w"], np.float32),
                          np.asarray(inputs["ln_b"], np.float32),
                          np.asarray(inputs["om_w"], np.float32),
                          np.asarray(inputs["om_b"], np.float32))
    return [dict(params, x=x[n]) for n in range(N)]


def run_traced(inputs):
    _install_patch()
    from concourse.bass_utils import run_bass_kernel_spmd
    if "nc" not in _cache:
        _cache["nc"] = _build_module()
    return run_bass_kernel_spmd(_cache["nc"], _prep_inputs(inputs),
                                core_ids=list(range(N)), trace=True)


# revision 48
# speedup vs baseline: 1.0683x; 1.0104x over previous
"""DCNv4-1D fused Trainium2 kernel v2. Data-parallel over batch N, 8 cores.

Per core (one sample), redesigned against the TimelineSim cost model:
  - x loaded via SWDGE cast-DMA f32->f16 (cost model charges OUT bytes).
  - LN stats: f16 ones-matmuls on PE into PSUM [33,512] (sum row 0, sumsq
    row 32); ACT copies to an f16 row; tiny DMAs repack to [128,w]; DVE/ACT
    finish rs/murs; DRAM round-trip broadcasts rs|murs to [128, 2*SC] f16.
    All Sqrt before any Gelu (2 ACT table loads total).
  - Apply: v = x16*rs_b, u = v - murs_b (DVE f16 TT, 2x mode), then
    xa = gelu(lnw*u + lnb) on ACT into xa_full with zeroed halo guards.
  - om = om_wT' @ xa on PE (f16), ACT adds bias(+grid fold), scatter DMAs
    land rows in qm_r [(g,s16), (o,f)] layout.
  - Banded conv (5 diagonals d=-2..2): coeff on DVE (tensor_scalar 4x via
    abs_max trick + TT mask/ksum); products coeff*xa on DVE f16 TT;
    d-sum mostly via PE id16 matmuls into PSUM + ACT copy-out, partly as
    DVE TT adds writing f16 directly (knob).
  - L split in 2 halves: half-0 conv overlaps half-1 front-end.
"""

import json

import numpy as np

N, C, L = 8, 256, 8192
G, K, GC = 8, 3, 32
LN_EPS = 1e-6
NCT = 2
NHALF = 2
LH = L // NHALF          # 4096
SC = 2048                # stats superchunk
NSC = L // SC            # 4 (2 per half)
SW = SC // 128           # 16 stats cols per partition per sc
CH = 256                 # conv chunk (16 per half -> partitions (g, s))
NCH_H = LH // CH         # 16
LQ = CH
HALO = 4
WIN = CH + 2 * HALO      # 264
D_LO, D_HI = -2, 2
ND = D_HI - D_LO + 1     # 5
CPG = 4                  # channels per product op
NCP = GC // CPG          # 4 product blocks per half

# ---- tuning knobs ----
DSUM_DVE_CP = ()         # cp blocks whose d-sum runs on DVE (rest PE+ACT)
OST_POOL_JP = ()         # (cp, jp) pairs whose PSUM->f16 copy runs on Pool
SCAT_SWDGE = True        # half the qm scatters on SWDGE (Pool) vs HWDGE

_cache = {}
_DBG_PSUM_INIT = False
_DBG_RELU = False


# --- BIR post-pass: this walrus build rejects >1 sync wait per instruction;
# split extras onto same-engine NoOps inserted just before the owner. ---
def _split_multi_waits(bir_json: bytes, max_waits: int = 1) -> bytes:
    j = json.loads(bir_json)
    n = [0]

    def fresh():
        n[0] += 1
        return f"I-wsplit-{n[0]}"

    for fn in j.get("functions", []):
        for bb in fn.get("basicblocks", []) or fn.get("blocks", []) or []:
            out = []
            for inst in bb.get("instructions", []):
                si = inst.get("sync_info")
                waits = (si or {}).get("on_wait") or []
                if len(waits) > max_waits:
                    for w in waits[:-max_waits]:
                        out.append({
                            "debug": inst.get("debug", 0),
                            "engine": inst["engine"],
                            "ins": [], "outs": [],
                            "name": fresh(),
                            "opcode": "NoOp",
                            "sync_info": {"on_update": [], "on_wait": [w]},
                        })
                    si["on_wait"] = waits[-max_waits:]
                out.append(inst)
            bb["instructions"] = out
    return json.dumps(j).encode()


def _install_patch():
    import os

    # The elide-DMA sem optimization assumes per-ring FIFO but DMAHW lanes
    # are shared across the SP/ACT rings; with our cross-ring interleave it
    # miscounts and consumers fire early. Disable it.
    os.environ["BACC_ELIDE_DMA_OPT_LIMIT"] = "0"
    import concourse.tile_sem_assignment as _tsa
    _tsa._opt_limit = 0

    import concourse.bass2jax as bass2jax
    import concourse.bass_utils as bass_utils

    if getattr(bass2jax.compile_bir_kernel, "_wsplit", False):
        return
    orig = bass_utils.compile_bir_kernel

    def patched(bir_json, tmpdir, neff_name="file.neff"):
        return orig(_split_multi_waits(bir_json), tmpdir, neff_name=neff_name)

    patched._wsplit = True
    bass_utils.compile_bir_kernel = patched
    bass2jax.compile_bir_kernel = patched


def _build_module():
    import contextlib

    import concourse.bass as bass
    import concourse.tile as tile
    from concourse import mybir

    f32 = mybir.dt.float32
    f16 = mybir.dt.float16
    AF = mybir.ActivationFunctionType
    GELU = AF.Relu if _DBG_RELU else AF.Gelu
    OP = mybir.AluOpType

    nc = bass.Bass()

    x_d = nc.dram_tensor("x", [C, L], f32, kind="ExternalInput")
    cst32_d = nc.dram_tensor("cst32", [128, 6], f32, kind="ExternalInput")
    cst16_d = nc.dram_tensor("cst16", [128, 226], f16, kind="ExternalInput")
    scr_d = nc.dram_tensor("scr", [NSC, 2 * SC], f16, kind="Internal")
    out_d = nc.dram_tensor("out", [C, L], f16, kind="ExternalOutput")

    with tile.TileContext(nc) as tc, contextlib.ExitStack() as ctx:
        const = ctx.enter_context(tc.tile_pool(name="const", bufs=1))
        persist = ctx.enter_context(tc.tile_pool(name="persist", bufs=1))
        xqp = ctx.enter_context(tc.tile_pool(name="xq", bufs=1))
        srp = ctx.enter_context(tc.tile_pool(name="srp", bufs=2))
        stp = ctx.enter_context(tc.tile_pool(name="stp", bufs=2))
        rbp = ctx.enter_context(tc.tile_pool(name="rbp", bufs=3))
        vup = ctx.enter_context(tc.tile_pool(name="vup", bufs=2))
        omp = ctx.enter_context(tc.tile_pool(name="omp", bufs=2))
        xrp = ctx.enter_context(tc.tile_pool(name="xrp", bufs=2))
        qmp = ctx.enter_context(tc.tile_pool(name="qmp", bufs=2))
        cfp = ctx.enter_context(tc.tile_pool(name="cfp", bufs=1))
        tmpp = ctx.enter_context(tc.tile_pool(name="tmpp", bufs=2))
        outp = ctx.enter_context(tc.tile_pool(name="outp", bufs=1))
        psum = ctx.enter_context(tc.tile_pool(name="psum", bufs=2,
                                              space="PSUM"))
        psum_o = ctx.enter_context(tc.tile_pool(name="psum_o", bufs=2,
                                                space="PSUM"))
        psum_a = ctx.enter_context(tc.tile_pool(name="psum_a", bufs=4,
                                                space="PSUM"))

        # HWDGE lane purity: DMAHW lanes are assigned round-robin over ALL
        # hwdge DMAs; threshold waits are only sound if each lane is fed by
        # one FIFO ring. Strict SP/ACT alternation keeps even lanes SP-only
        # and odd lanes ACT-only (NUM_HWDGE_SEMS == 8).
        _hwflip = [0]

        def hw_dma(out, in_):
            eng = nc.sync if _hwflip[0] % 2 == 0 else nc.scalar
            _hwflip[0] += 1
            eng.dma_start(out=out, in_=in_)

        # ---------------- constants ----------------
        cst32 = const.tile([128, 6], f32, tag="cst32", name="cst32")
        hw_dma(out=cst32, in_=cst32_d[:])
        cst16 = const.tile([128, 226], f16, tag="cst16", name="cst16")
        hw_dma(out=cst16, in_=cst16_d[:])
        lnw_c = [cst32[:, ct:ct + 1] for ct in range(NCT)]
        lnb_c = [cst32[:, 2 + ct:3 + ct] for ct in range(NCT)]
        bias48 = cst32[0:48, 5:6]
        omwT = [cst16[:, ct * 48:(ct + 1) * 48] for ct in range(NCT)]
        id16 = cst16[:, 96:224]
        onesc16 = cst16[:, 224:225]
        eps_c = const.tile([128, 1], f32, tag="eps", name="eps")
        nc.vector.memset(eps_c, LN_EPS)

        # ---------------- persistent tensors ----------------
        x16 = [persist.tile([128, L], f16, tag=f"x16_{ct}",
                            name=f"x16_{ct}") for ct in range(NCT)]
        xa_full = [persist.tile([128, 2 * HALO + L], f16, tag=f"xaf{ct}",
                                name=f"xaf{ct}") for ct in range(NCT)]
        for ct in range(NCT):
            nc.vector.memset(xa_full[ct][:, 0:HALO], 0.0)
            nc.vector.memset(xa_full[ct][:, HALO + L:2 * HALO + L], 0.0)

        # x cast loads (SWDGE), chunked per superchunk so stats start early
        for sc in range(NSC):
            for ct in range(NCT):
                nc.gpsimd.dma_start(
                    out=x16[ct][:, sc * SC:(sc + 1) * SC],
                    in_=x_d[ct * 128:(ct + 1) * 128, sc * SC:(sc + 1) * SC])

        rb_l = [None] * NSC

        # ============ phase 1: LN stats, staged across superchunks ============
        # (same-stage DMAs are adjacent in each queue so the four superchunk
        # round-trips pipeline instead of serializing end-to-end)
        spk_l, qpk_l, rsmu_l = [], [], []
        for sc in range(NSC):
            lo = sc * SC
            srow = srp.tile([33, SC], f16, tag="srow", name="srow")
            xq = [None] * NCT
            for ct in range(NCT):
                xq[ct] = xqp.tile([128, SC], f16, tag=f"xq{ct}",
                                  name=f"xq{ct}")
                nc.vector.tensor_tensor(out=xq[ct],
                                        in0=x16[ct][:, lo:lo + SC],
                                        in1=x16[ct][:, lo:lo + SC],
                                        op=OP.mult)
            for c in range(SC // 512):
                cf = c * 512
                st_ps = psum.tile([33, 512], f32, tag="st", name="st")
                if _DBG_PSUM_INIT:
                    nc.vector.memset(st_ps[1:32, :], 0.0)
                for ct in range(NCT):
                    nc.tensor.matmul(st_ps[0:1, :], onesc16,
                                     x16[ct][:, lo + cf:lo + cf + 512],
                                     start=(ct == 0), stop=(ct == NCT - 1))
                for ct in range(NCT):
                    nc.tensor.matmul(st_ps[32:33, :], onesc16,
                                     xq[ct][:, cf:cf + 512],
                                     start=(ct == 0), stop=(ct == NCT - 1))
                nc.scalar.copy(out=srow[:, cf:cf + 512], in_=st_ps)

            spk = stp.tile([128, SW], f16, tag=f"spk{sc}", name=f"spk{sc}")
            hw_dma(
                out=spk,
                in_=srow[0:1, :].rearrange("one (p w) -> one p w", p=128))
            qpk = stp.tile([128, SW], f16, tag=f"qpk{sc}", name=f"qpk{sc}")
            hw_dma(
                out=qpk,
                in_=srow[32:33, :].rearrange("one (p w) -> one p w", p=128))
            spk_l.append(spk)
            qpk_l.append(qpk)

        for sc in range(NSC):
            spk, qpk = spk_l[sc], qpk_l[sc]
            mu_t = stp.tile([128, SW], f32, tag="mu_t", name="mu_t")
            nc.vector.tensor_scalar_mul(out=mu_t, in0=spk, scalar1=1.0 / C)
            musq = stp.tile([128, SW], f32, tag="musq", name="musq")
            nc.vector.tensor_tensor(out=musq, in0=mu_t, in1=mu_t,
                                    op=OP.mult)
            varq = stp.tile([128, SW], f32, tag="varq", name="varq")
            nc.vector.scalar_tensor_tensor(out=varq, in0=qpk, scalar=1.0 / C,
                                           in1=musq, op0=OP.mult,
                                           op1=OP.subtract)
            sd = stp.tile([128, SW], f32, tag="sd", name="sd")
            nc.scalar.activation(out=sd, in_=varq, func=AF.Sqrt,
                                 bias=eps_c, scale=1.0)
            rs32 = stp.tile([128, SW], f32, tag="rs32", name="rs32")
            nc.vector.reciprocal(out=rs32, in_=sd)
            rsmu = stp.tile([128, 2 * SW], f16, tag=f"rsmu{sc}",
                            name=f"rsmu{sc}")
            nc.vector.tensor_scalar_mul(out=rsmu[:, 0:SW], in0=rs32,
                                        scalar1=1.0)
            nc.vector.scalar_tensor_tensor(out=rsmu[:, SW:2 * SW], in0=spk,
                                           scalar=1.0 / C, in1=rs32,
                                           op0=OP.mult, op1=OP.mult)
            rsmu_l.append(rsmu)
            nc.gpsimd.dma_start(
                out=scr_d[sc:sc + 1, :].rearrange("one (p w) -> one p w",
                                                  p=128),
                in_=rsmu)

        for sc in range(NSC):
            halves = []
            row = scr_d[sc:sc + 1, :]
            for rh in range(2):
                rb = rbp.tile([128, SC], f16, tag=f"rb{rh}", name="rb")
                hw_dma(
                    out=rb,
                    in_=bass.AP(tensor=row.tensor,
                                offset=row.offset + rh * SC,
                                ap=[[0, 128], [1, SC]]))
                halves.append(rb)
            rb_l[sc] = halves

        # per-half state
        qm_r = [None] * NHALF
        xa_r = [None] * NHALF
        prev_hi = {0: -1, 1: -1}

        def windows_dma(h, s, ct, dma):
            # xa window for conv chunk s of half h, channel tile ct
            st = h * LH + s * CH
            dma(
                out=xa_r[h][ct * 64 + s:ct * 64 + s + 49:16, :]
                .rearrange("g (cc w) -> g cc w", cc=GC),
                in_=xa_full[ct][:, st:st + WIN])

        def flush_windows(h, s_hi):
            # inline: only SWDGE windows (HWDGE-ring ones would stall the
            # issuing engine's SEQ between gelus); HWDGE half is deferred
            # to emit_ph2b once the half's gelus are all emitted.
            s_hi = min(s_hi, NCH_H - 1)
            for s in range(prev_hi[h] + 1, s_hi + 1):
                for ct in range(NCT):
                    if (s + ct) % 2 == 1:
                        windows_dma(h, s, ct, nc.gpsimd.dma_start)
            prev_hi[h] = max(prev_hi[h], s_hi)

        def flush_windows_hw(h):
            for s in range(NCH_H):
                for ct in range(NCT):
                    if (s + ct) % 2 == 0:
                        windows_dma(h, s, ct, hw_dma)

        # ============= phase 2: apply + gelu + om (per half) =============
        APW = 1024  # apply op width
        SC_PER_H = LH // SC  # 2

        def emit_apply(sc, lo_off, width):
            # LN apply + gelu for x columns [sc*SC+lo_off, +width)
            rh = (lo_off // SW) // 64
            rb_v = rb_l[sc][rh][:].rearrange("q (p half w) -> q p half w",
                                             p=64, half=2)
            rs_sl = rb_v[:, :, 0, :]
            mu_sl = rb_v[:, :, 1, :]
            lo = sc * SC + lo_off
            pw = width // SW
            p0 = (lo_off // SW) % 64
            for ct in range(NCT):
                xs = x16[ct][:, lo:lo + width].rearrange(
                    "p (a w) -> p a w", a=pw)
                v = vup.tile([128, APW], f16, tag=f"v{ct}", name=f"v{ct}")
                vv = v[:, 0:width].rearrange("p (a w) -> p a w", a=pw)
                nc.vector.tensor_tensor(
                    out=vv, in0=xs, in1=rs_sl[:, p0:p0 + pw, :], op=OP.mult)
                nc.vector.tensor_tensor(
                    out=vv, in0=vv, in1=mu_sl[:, p0:p0 + pw, :],
                    op=OP.subtract)
                nc.scalar.activation(
                    out=xa_full[ct][:, HALO + lo:HALO + lo + width],
                    in_=v[:, 0:width], func=GELU, bias=lnb_c[ct],
                    scale=lnw_c[ct])

        def emit_ph2a(h):
            xa_r[h] = xrp.tile([128, GC * WIN], f16, tag="xa_r",
                               name=f"xa_r{h}")
            for sc_h in range(SC_PER_H):
                sc = h * SC_PER_H + sc_h
                for a2 in range(SC // APW):
                    emit_apply(sc, a2 * APW, APW)
                    cov = sc_h * SC + (a2 + 1) * APW
                    flush_windows(h, (cov - CH - HALO) // CH)
            if h == 0:
                # pre-compute h1's first 512 cols so h0's last window (and
                # thus h0's conv) doesn't wait for h1's front-end. h1
                # recomputes the same values later (idempotent).
                emit_apply(SC_PER_H, 0, 512)
                flush_windows(h, NCH_H - 1)

        def emit_ph2b(h):
            qm_r[h] = qmp.tile([128, 2 * K * LQ], f16, tag="qm_r",
                               name=f"qm_r{h}")
            if h == NHALF - 1:
                flush_windows(h, NCH_H - 1)
            flush_windows_hw(h)
            for gp in range(LH // 512):
                lo5 = h * LH + gp * 512
                om_ps = psum_o.tile([48, 512], f32, tag="om", name="om")
                for ct in range(NCT):
                    nc.tensor.matmul(
                        om_ps, omwT[ct],
                        xa_full[ct][:, HALO + lo5:HALO + lo5 + 512],
                        start=(ct == 0), stop=(ct == NCT - 1))
                om_st = omp.tile([48, 512], f16, tag="om_st", name="om_st")
                nc.scalar.activation(out=om_st, in_=om_ps,
                                     func=AF.Identity, bias=bias48,
                                     scale=1.0)
                for s2 in range(2):
                    s = gp * 2 + s2  # conv chunk within half
                    hw_dma(
                        out=qm_r[h][s:128:16, :]
                        .rearrange("g (o f) -> g o f", o=6),
                        in_=om_st[:, s2 * 256:(s2 + 1) * 256])

        # ================= phase 3: banded conv per half =================
        def emit_ph3(h):
            q_ap = qm_r[h][:, 0:K * LQ]
            m_ap = qm_r[h][:, K * LQ:2 * K * LQ]
            hm_all = cfp.tile([128, ND * K * LQ], f16, tag="hm",
                              name=f"hm{h}")
            r2t = cfp.tile([128, K * LQ], f16, tag="r2t", name="r2t")
            for i in range(ND):
                d = D_LO + i
                sl = hm_all[:, i * K * LQ:(i + 1) * K * LQ]
                nc.vector.tensor_scalar_add(out=sl, in0=q_ap,
                                            scalar1=float(1 - d))
                nc.vector.tensor_scalar(out=r2t, in0=q_ap,
                                        scalar1=float(d + 1), scalar2=-1.0,
                                        op0=OP.subtract, op1=OP.mult)
                nc.vector.tensor_tensor(out=sl, in0=sl, in1=r2t, op=OP.min)
                nc.vector.tensor_scalar_max(out=sl, in0=sl, scalar1=0.0)
            m_rep = bass.AP(tensor=m_ap.tensor, offset=m_ap.offset,
                            ap=[m_ap.ap[0], [0, ND], [1, K * LQ]])
            nc.vector.tensor_tensor(
                out=hm_all[:].rearrange("p (d f) -> p d f", d=ND),
                in0=hm_all[:].rearrange("p (d f) -> p d f", d=ND),
                in1=m_rep, op=OP.mult)
            cd_all = cfp.tile([128, ND * LQ], f16, tag="cd", name=f"cd{h}")
            hm_v = hm_all[:].rearrange("p (d k f) -> p d k f", d=ND, k=K)
            cd_v = cd_all[:].rearrange("p (d f) -> p d f", d=ND)
            nc.vector.tensor_tensor(out=cd_v, in0=hm_v[:, :, 0, :],
                                    in1=hm_v[:, :, 1, :], op=OP.add)
            nc.vector.tensor_tensor(out=cd_v, in0=cd_v,
                                    in1=hm_v[:, :, 2, :], op=OP.add)

            xa_r_v = xa_r[h][:].rearrange("p (c wn) -> p c wn", c=GC)
            out_t = out_d[:]
            ost_all = outp.tile([128, GC * LQ], f16, tag="ost_all",
                                name=f"ost_all{h}")
            for cp in range(NCP):
                tmps = []
                for i in range(ND):
                    d = D_LO + i
                    tmp = tmpp.tile([128, CPG * LQ], f16, tag=f"tmp{i}",
                                    name=f"tmp{i}")
                    cdi = cd_all[:, i * LQ:(i + 1) * LQ]
                    cb = bass.AP(tensor=cdi.tensor, offset=cdi.offset,
                                 ap=[cdi.ap[0], [0, CPG], [1, LQ]])
                    # Pool is idle in the tail; let it take a slice of the
                    # last half's products to shorten the DVE-bound tail.
                    peng = (nc.gpsimd if (h == 1 and cp >= 5 and i == 4)
                            else nc.vector)
                    peng.tensor_tensor(
                        out=tmp[:].rearrange("p (c f) -> p c f", c=CPG),
                        in0=xa_r_v[:, cp * CPG:(cp + 1) * CPG,
                                   HALO + d:HALO + d + LQ],
                        in1=cb, op=OP.mult)
                    tmps.append(tmp)
                o0 = cp * CPG * LQ
                if cp in DSUM_DVE_CP:
                    nc.vector.tensor_tensor(out=tmps[0], in0=tmps[0],
                                            in1=tmps[1], op=OP.add)
                    nc.vector.tensor_tensor(out=tmps[2], in0=tmps[2],
                                            in1=tmps[3], op=OP.add)
                    nc.vector.tensor_tensor(out=tmps[0], in0=tmps[0],
                                            in1=tmps[2], op=OP.add)
                    nc.vector.tensor_tensor(
                        out=ost_all[:, o0:o0 + CPG * LQ], in0=tmps[0],
                        in1=tmps[4], op=OP.add)
                else:
                    for jp in range(CPG // 2):
                        acc = psum_a.tile([128, 2 * LQ], f32, tag="acc",
                                          name="acc")
                        for jj in range(2):
                            j = jp * 2 + jj
                            for n_i in range(ND):
                                nc.tensor.matmul(
                                    acc[:, jj * LQ:(jj + 1) * LQ], id16,
                                    tmps[n_i][:, j * LQ:(j + 1) * LQ],
                                    start=(n_i == 0), stop=(n_i == ND - 1))
                        oj = o0 + jp * 2 * LQ
                        if (cp, jp) in OST_POOL_JP:
                            nc.gpsimd.tensor_copy(
                                out=ost_all[:, oj:oj + 2 * LQ], in_=acc)
                        else:
                            nc.scalar.copy(
                                out=ost_all[:, oj:oj + 2 * LQ], in_=acc)
            for g in range(G):
                hw_dma(
                    out=bass.AP(
                        tensor=out_t.tensor,
                        offset=out_t.offset + g * GC * L + h * LH,
                        ap=[[CH, NCH_H], [L, GC], [1, LQ]]),
                    in_=ost_all[g * 16:g * 16 + 16, :]
                    .rearrange("s (c f) -> s c f", c=GC))

        # driver: front-ends first (conv overlaps them via per-engine
        # in-order streams)
        emit_ph2a(0)
        emit_ph2a(1)
        emit_ph2b(0)
        emit_ph2b(1)
        emit_ph3(0)
        emit_ph3(1)

    return nc


def _prep_params(ln_w, ln_b, om_w, om_b):
    cst32 = np.zeros((128, 6), np.float32)
    cst32[:, 0] = ln_w[0:128]
    cst32[:, 1] = ln_w[128:256]
    cst32[:, 2] = ln_b[0:128]
    cst32[:, 3] = ln_b[128:256]
    cst32[:, 4] = 1.0
    # bias48 in (g, qm, k) order with conv grid fold for qm=0 (offsets)
    for g in range(G):
        for k in range(K):
            cst32[g * 6 + k, 5] = om_b[g * K + k] + (k - 1.0)
            cst32[g * 6 + 3 + k, 5] = om_b[G * K + g * K + k]
    cst16 = np.zeros((128, 226), np.float16)
    omwT = om_w.T.astype(np.float16)          # [C, 48] original row order
    perm = np.zeros(48, np.int64)
    for g in range(G):
        for k in range(K):
            perm[g * 6 + k] = g * K + k
            perm[g * 6 + 3 + k] = G * K + g * K + k
    omwTp = omwT[:, perm]                      # columns in (g,qm,k) order
    cst16[:, 0:48] = omwTp[0:128]
    cst16[:, 48:96] = omwTp[128:256]
    cst16[:, 96:224] = np.eye(128, dtype=np.float16)
    cst16[:, 224] = 1.0
    return {"cst32": cst32, "cst16": cst16}


def kernel(x, ln_w, ln_b, om_w, om_b):
    _install_patch()
    from concourse.bass_utils import run_bass_kernel_spmd

    if "nc" not in _cache:
        _cache["nc"] = _build_module()
    nc = _cache["nc"]

    x = np.ascontiguousarray(np.asarray(x, dtype=np.float32))
    params = _prep_params(np.asarray(ln_w, np.float32),
                          np.asarray(ln_b, np.float32),
                          np.asarray(om_w, np.float32),
                          np.asarray(om_b, np.float32))
    in_maps = [dict(params, x=x[n]) for n in range(N)]
    res = run_bass_kernel_spmd(nc, in_maps, core_ids=list(range(N)))
    return np.stack([res.results[n]["out"] for n in range(N)],
                    axis=0).astype(np.float32)


def _prep_inputs(inputs):
    x = np.ascontiguousarray(np.asarray(inputs["x"], dtype=np.float32))
    params = _prep_params(np.asarray(inputs["ln_w"], np.float32),
                          np.asarray(inputs["ln_b"], np.float32),
                          np.asarray(inputs["om_w"], np.float32),
                          np.asarray(inputs["om_b"], np.float32))
    return [dict(params, x=x[n]) for n in range(N)]


def run_traced(inputs):
    _install_patch()
    from concourse.bass_utils import run_bass_kernel_spmd
    if "nc" not in _cache:
        _cache["nc"] = _build_module()
    return run_bass_kernel_spmd(_cache["nc"], _prep_inputs(inputs),
                                core_ids=list(range(N)), trace=True)


# revision 60
# speedup vs baseline: 1.0736x; 1.0049x over previous
"""DCNv4-1D fused Trainium2 kernel v2. Data-parallel over batch N, 8 cores.

Per core (one sample), redesigned against the TimelineSim cost model:
  - x loaded via SWDGE cast-DMA f32->f16 (cost model charges OUT bytes).
  - LN stats: f16 ones-matmuls on PE into PSUM [33,512] (sum row 0, sumsq
    row 32); ACT copies to an f16 row; tiny DMAs repack to [128,w]; DVE/ACT
    finish rs/murs; DRAM round-trip broadcasts rs|murs to [128, 2*SC] f16.
    All Sqrt before any Gelu (2 ACT table loads total).
  - Apply: v = x16*rs_b, u = v - murs_b (DVE f16 TT, 2x mode), then
    xa = gelu(lnw*u + lnb) on ACT into xa_full with zeroed halo guards.
  - om = om_wT' @ xa on PE (f16), ACT adds bias(+grid fold), scatter DMAs
    land rows in qm_r [(g,s16), (o,f)] layout.
  - Banded conv (5 diagonals d=-2..2): coeff on DVE (tensor_scalar 4x via
    abs_max trick + TT mask/ksum); products coeff*xa on DVE f16 TT;
    d-sum mostly via PE id16 matmuls into PSUM + ACT copy-out, partly as
    DVE TT adds writing f16 directly (knob).
  - L split in 2 halves: half-0 conv overlaps half-1 front-end.
"""

import json

import numpy as np

N, C, L = 8, 256, 8192
G, K, GC = 8, 3, 32
LN_EPS = 1e-6
NCT = 2
NHALF = 2
LH = L // NHALF          # 4096
SC = 2048                # stats superchunk
NSC = L // SC            # 4 (2 per half)
SW = SC // 128           # 16 stats cols per partition per sc
CH = 256                 # conv chunk (16 per half -> partitions (g, s))
NCH_H = LH // CH         # 16
LQ = CH
HALO = 4
WIN = CH + 2 * HALO      # 264
D_LO, D_HI = -2, 2
ND = D_HI - D_LO + 1     # 5
CPG = 4                  # channels per product op
NCP = GC // CPG          # 4 product blocks per half

# ---- tuning knobs ----
DSUM_DVE_CP = ()         # cp blocks whose d-sum runs on DVE (rest PE+ACT)
OST_POOL_JP = ()         # (cp, jp) pairs whose PSUM->f16 copy runs on Pool
SCAT_SWDGE = True        # half the qm scatters on SWDGE (Pool) vs HWDGE

_cache = {}
_DBG_PSUM_INIT = False
_DBG_RELU = False


# --- BIR post-pass: this walrus build rejects >1 sync wait per instruction;
# split extras onto same-engine NoOps inserted just before the owner. ---
def _split_multi_waits(bir_json: bytes, max_waits: int = 1) -> bytes:
    j = json.loads(bir_json)
    n = [0]

    def fresh():
        n[0] += 1
        return f"I-wsplit-{n[0]}"

    for fn in j.get("functions", []):
        for bb in fn.get("basicblocks", []) or fn.get("blocks", []) or []:
            out = []
            for inst in bb.get("instructions", []):
                si = inst.get("sync_info")
                waits = (si or {}).get("on_wait") or []
                if len(waits) > max_waits:
                    for w in waits[:-max_waits]:
                        out.append({
                            "debug": inst.get("debug", 0),
                            "engine": inst["engine"],
                            "ins": [], "outs": [],
                            "name": fresh(),
                            "opcode": "NoOp",
                            "sync_info": {"on_update": [], "on_wait": [w]},
                        })
                    si["on_wait"] = waits[-max_waits:]
                out.append(inst)
            bb["instructions"] = out
    return json.dumps(j).encode()


def _install_patch():
    import os

    # The elide-DMA sem optimization assumes per-ring FIFO but DMAHW lanes
    # are shared across the SP/ACT rings; with our cross-ring interleave it
    # miscounts and consumers fire early. Disable it.
    os.environ["BACC_ELIDE_DMA_OPT_LIMIT"] = "0"
    import concourse.tile_sem_assignment as _tsa
    _tsa._opt_limit = 0

    import concourse.bass2jax as bass2jax
    import concourse.bass_utils as bass_utils

    if getattr(bass2jax.compile_bir_kernel, "_wsplit", False):
        return
    orig = bass_utils.compile_bir_kernel

    def patched(bir_json, tmpdir, neff_name="file.neff"):
        return orig(_split_multi_waits(bir_json), tmpdir, neff_name=neff_name)

    patched._wsplit = True
    bass_utils.compile_bir_kernel = patched
    bass2jax.compile_bir_kernel = patched


def _build_module():
    import contextlib

    import concourse.bass as bass
    import concourse.tile as tile
    from concourse import mybir

    f32 = mybir.dt.float32
    f16 = mybir.dt.float16
    AF = mybir.ActivationFunctionType
    GELU = AF.Relu if _DBG_RELU else AF.Gelu
    OP = mybir.AluOpType

    nc = bass.Bass()

    x_d = nc.dram_tensor("x", [C, L], f32, kind="ExternalInput")
    cst32_d = nc.dram_tensor("cst32", [128, 6], f32, kind="ExternalInput")
    cst16_d = nc.dram_tensor("cst16", [128, 226], f16, kind="ExternalInput")
    scr_d = nc.dram_tensor("scr", [NSC, 2 * SC], f16, kind="Internal")
    out_d = nc.dram_tensor("out", [C, L], f16, kind="ExternalOutput")

    with tile.TileContext(nc) as tc, contextlib.ExitStack() as ctx:
        const = ctx.enter_context(tc.tile_pool(name="const", bufs=1))
        persist = ctx.enter_context(tc.tile_pool(name="persist", bufs=1))
        xqp = ctx.enter_context(tc.tile_pool(name="xq", bufs=1))
        srp = ctx.enter_context(tc.tile_pool(name="srp", bufs=2))
        stp = ctx.enter_context(tc.tile_pool(name="stp", bufs=2))
        rbp = ctx.enter_context(tc.tile_pool(name="rbp", bufs=3))
        vup = ctx.enter_context(tc.tile_pool(name="vup", bufs=2))
        omp = ctx.enter_context(tc.tile_pool(name="omp", bufs=2))
        xrp = ctx.enter_context(tc.tile_pool(name="xrp", bufs=2))
        qmp = ctx.enter_context(tc.tile_pool(name="qmp", bufs=2))
        cfp = ctx.enter_context(tc.tile_pool(name="cfp", bufs=1))
        tmpp = ctx.enter_context(tc.tile_pool(name="tmpp", bufs=2))
        outp = ctx.enter_context(tc.tile_pool(name="outp", bufs=1))
        psum = ctx.enter_context(tc.tile_pool(name="psum", bufs=2,
                                              space="PSUM"))
        psum_o = ctx.enter_context(tc.tile_pool(name="psum_o", bufs=2,
                                                space="PSUM"))
        psum_a = ctx.enter_context(tc.tile_pool(name="psum_a", bufs=4,
                                                space="PSUM"))

        # HWDGE lane purity: DMAHW lanes are assigned round-robin over ALL
        # hwdge DMAs; threshold waits are only sound if each lane is fed by
        # one FIFO ring. Strict SP/ACT alternation keeps even lanes SP-only
        # and odd lanes ACT-only (NUM_HWDGE_SEMS == 8).
        _hwflip = [0]

        def hw_dma(out, in_):
            eng = nc.sync if _hwflip[0] % 2 == 0 else nc.scalar
            _hwflip[0] += 1
            eng.dma_start(out=out, in_=in_)

        # ---------------- constants ----------------
        cst32 = const.tile([128, 6], f32, tag="cst32", name="cst32")
        hw_dma(out=cst32, in_=cst32_d[:])
        cst16 = const.tile([128, 226], f16, tag="cst16", name="cst16")
        hw_dma(out=cst16, in_=cst16_d[:])
        lnw_c = [cst32[:, ct:ct + 1] for ct in range(NCT)]
        lnb_c = [cst32[:, 2 + ct:3 + ct] for ct in range(NCT)]
        bias48 = cst32[0:48, 5:6]
        omwT = [cst16[:, ct * 48:(ct + 1) * 48] for ct in range(NCT)]
        id16 = cst16[:, 96:224]
        onesc16 = cst16[:, 224:225]
        eps_c = const.tile([128, 1], f32, tag="eps", name="eps")
        nc.vector.memset(eps_c, LN_EPS)

        # ---------------- persistent tensors ----------------
        x16 = [persist.tile([128, L], f16, tag=f"x16_{ct}",
                            name=f"x16_{ct}") for ct in range(NCT)]
        xa_full = [persist.tile([128, 2 * HALO + L], f16, tag=f"xaf{ct}",
                                name=f"xaf{ct}") for ct in range(NCT)]
        for ct in range(NCT):
            nc.vector.memset(xa_full[ct][:, 0:HALO], 0.0)
            nc.vector.memset(xa_full[ct][:, HALO + L:2 * HALO + L], 0.0)

        # x cast loads (SWDGE), chunked per superchunk so stats start early
        for sc in range(NSC):
            for ct in range(NCT):
                nc.gpsimd.dma_start(
                    out=x16[ct][:, sc * SC:(sc + 1) * SC],
                    in_=x_d[ct * 128:(ct + 1) * 128, sc * SC:(sc + 1) * SC])

        rb_l = [None] * NSC

        # ============ phase 1: LN stats, staged across superchunks ============
        # (same-stage DMAs are adjacent in each queue so the four superchunk
        # round-trips pipeline instead of serializing end-to-end)
        spk_l, qpk_l, rsmu_l = [], [], []
        for sc in range(NSC):
            lo = sc * SC
            srow = srp.tile([33, SC], f16, tag="srow", name="srow")
            xq = [None] * NCT
            for ct in range(NCT):
                xq[ct] = xqp.tile([128, SC], f16, tag=f"xq{ct}",
                                  name=f"xq{ct}")
                nc.vector.tensor_tensor(out=xq[ct],
                                        in0=x16[ct][:, lo:lo + SC],
                                        in1=x16[ct][:, lo:lo + SC],
                                        op=OP.mult)
            for c in range(SC // 512):
                cf = c * 512
                st_ps = psum.tile([33, 512], f32, tag="st", name="st")
                if _DBG_PSUM_INIT:
                    nc.vector.memset(st_ps[1:32, :], 0.0)
                for ct in range(NCT):
                    nc.tensor.matmul(st_ps[0:1, :], onesc16,
                                     x16[ct][:, lo + cf:lo + cf + 512],
                                     start=(ct == 0), stop=(ct == NCT - 1))
                for ct in range(NCT):
                    nc.tensor.matmul(st_ps[32:33, :], onesc16,
                                     xq[ct][:, cf:cf + 512],
                                     start=(ct == 0), stop=(ct == NCT - 1))
                nc.scalar.copy(out=srow[:, cf:cf + 512], in_=st_ps)

            spk = stp.tile([128, SW], f16, tag=f"spk{sc}", name=f"spk{sc}")
            hw_dma(
                out=spk,
                in_=srow[0:1, :].rearrange("one (p w) -> one p w", p=128))
            qpk = stp.tile([128, SW], f16, tag=f"qpk{sc}", name=f"qpk{sc}")
            hw_dma(
                out=qpk,
                in_=srow[32:33, :].rearrange("one (p w) -> one p w", p=128))
            spk_l.append(spk)
            qpk_l.append(qpk)

        for sc in range(NSC):
            spk, qpk = spk_l[sc], qpk_l[sc]
            mu_t = stp.tile([128, SW], f32, tag="mu_t", name="mu_t")
            nc.vector.tensor_scalar_mul(out=mu_t, in0=spk, scalar1=1.0 / C)
            musq = stp.tile([128, SW], f32, tag="musq", name="musq")
            nc.vector.tensor_tensor(out=musq, in0=mu_t, in1=mu_t,
                                    op=OP.mult)
            varq = stp.tile([128, SW], f32, tag="varq", name="varq")
            nc.vector.scalar_tensor_tensor(out=varq, in0=qpk, scalar=1.0 / C,
                                           in1=musq, op0=OP.mult,
                                           op1=OP.subtract)
            sd = stp.tile([128, SW], f32, tag="sd", name="sd")
            nc.scalar.activation(out=sd, in_=varq, func=AF.Sqrt,
                                 bias=eps_c, scale=1.0)
            rs32 = stp.tile([128, SW], f32, tag="rs32", name="rs32")
            nc.vector.reciprocal(out=rs32, in_=sd)
            rsmu = stp.tile([128, 2 * SW], f16, tag=f"rsmu{sc}",
                            name=f"rsmu{sc}")
            nc.vector.tensor_scalar_mul(out=rsmu[:, 0:SW], in0=rs32,
                                        scalar1=1.0)
            nc.vector.scalar_tensor_tensor(out=rsmu[:, SW:2 * SW], in0=spk,
                                           scalar=1.0 / C, in1=rs32,
                                           op0=OP.mult, op1=OP.mult)
            rsmu_l.append(rsmu)
            nc.gpsimd.dma_start(
                out=scr_d[sc:sc + 1, :].rearrange("one (p w) -> one p w",
                                                  p=128),
                in_=rsmu)

        for sc in range(NSC):
            halves = []
            row = scr_d[sc:sc + 1, :]
            for rh in range(2):
                rb = rbp.tile([128, SC], f16, tag=f"rb{rh}", name="rb")
                hw_dma(
                    out=rb,
                    in_=bass.AP(tensor=row.tensor,
                                offset=row.offset + rh * SC,
                                ap=[[0, 128], [1, SC]]))
                halves.append(rb)
            rb_l[sc] = halves

        # per-half state
        qm_r = [None] * NHALF
        xa_r = [None] * NHALF
        prev_hi = {0: -1, 1: -1}

        def windows_dma(h, s, ct, dma):
            # xa window for conv chunk s of half h, channel tile ct
            st = h * LH + s * CH
            dma(
                out=xa_r[h][ct * 64 + s:ct * 64 + s + 49:16, :]
                .rearrange("g (cc w) -> g cc w", cc=GC),
                in_=xa_full[ct][:, st:st + WIN])

        def flush_windows(h, s_hi):
            # inline: only SWDGE windows (HWDGE-ring ones would stall the
            # issuing engine's SEQ between gelus); HWDGE half is deferred
            # to emit_ph2b once the half's gelus are all emitted.
            s_hi = min(s_hi, NCH_H - 1)
            for s in range(prev_hi[h] + 1, s_hi + 1):
                for ct in range(NCT):
                    if (s + ct) % 2 == 1:
                        windows_dma(h, s, ct, nc.gpsimd.dma_start)
            prev_hi[h] = max(prev_hi[h], s_hi)

        def flush_windows_hw(h):
            for s in range(NCH_H):
                for ct in range(NCT):
                    if (s + ct) % 2 == 0:
                        windows_dma(h, s, ct, hw_dma)

        # ============= phase 2: apply + gelu + om (per half) =============
        APW = 1024  # apply op width
        SC_PER_H = LH // SC  # 2

        def emit_apply(sc, lo_off, width):
            # LN apply + gelu for x columns [sc*SC+lo_off, +width)
            rh = (lo_off // SW) // 64
            rb_v = rb_l[sc][rh][:].rearrange("q (p half w) -> q p half w",
                                             p=64, half=2)
            rs_sl = rb_v[:, :, 0, :]
            mu_sl = rb_v[:, :, 1, :]
            lo = sc * SC + lo_off
            pw = width // SW
            p0 = (lo_off // SW) % 64
            for ct in range(NCT):
                xs = x16[ct][:, lo:lo + width].rearrange(
                    "p (a w) -> p a w", a=pw)
                v = vup.tile([128, APW], f16, tag=f"v{ct}", name=f"v{ct}")
                vv = v[:, 0:width].rearrange("p (a w) -> p a w", a=pw)
                nc.vector.tensor_tensor(
                    out=vv, in0=xs, in1=rs_sl[:, p0:p0 + pw, :], op=OP.mult)
                nc.vector.tensor_tensor(
                    out=vv, in0=vv, in1=mu_sl[:, p0:p0 + pw, :],
                    op=OP.subtract)
                nc.scalar.activation(
                    out=xa_full[ct][:, HALO + lo:HALO + lo + width],
                    in_=v[:, 0:width], func=GELU, bias=lnb_c[ct],
                    scale=lnw_c[ct])

        def emit_ph2a(h):
            xa_r[h] = xrp.tile([128, GC * WIN], f16, tag="xa_r",
                               name=f"xa_r{h}")
            for sc_h in range(SC_PER_H):
                sc = h * SC_PER_H + sc_h
                for a2 in range(SC // APW):
                    emit_apply(sc, a2 * APW, APW)
                    cov = sc_h * SC + (a2 + 1) * APW
                    flush_windows(h, (cov - CH - HALO) // CH)
            if h == 0:
                # pre-compute h1's first 512 cols so h0's last window (and
                # thus h0's conv) doesn't wait for h1's front-end. h1
                # recomputes the same values later (idempotent).
                emit_apply(SC_PER_H, 0, 512)
                flush_windows(h, NCH_H - 1)

        def emit_ph2b(h):
            qm_r[h] = qmp.tile([128, 2 * K * LQ], f16, tag="qm_r",
                               name=f"qm_r{h}")
            if h == NHALF - 1:
                flush_windows(h, NCH_H - 1)
            flush_windows_hw(h)
            for gp in range(LH // 512):
                lo5 = h * LH + gp * 512
                om_ps = psum_o.tile([48, 512], f32, tag="om", name="om")
                for ct in range(NCT):
                    nc.tensor.matmul(
                        om_ps, omwT[ct],
                        xa_full[ct][:, HALO + lo5:HALO + lo5 + 512],
                        start=(ct == 0), stop=(ct == NCT - 1))
                om_st = omp.tile([48, 512], f16, tag="om_st", name="om_st")
                if h == 0:
                    # DVE is stalled waiting these scatters anyway; doing the
                    # bias+convert there skips the ACT queue (which is still
                    # busy with h1's gelus) and feeds coeff(h0) ~10us earlier
                    nc.vector.tensor_scalar_add(out=om_st, in0=om_ps,
                                                scalar1=bias48)
                else:
                    nc.scalar.activation(out=om_st, in_=om_ps,
                                         func=AF.Identity, bias=bias48,
                                         scale=1.0)
                for s2 in range(2):
                    s = gp * 2 + s2  # conv chunk within half
                    hw_dma(
                        out=qm_r[h][s:128:16, :]
                        .rearrange("g (o f) -> g o f", o=6),
                        in_=om_st[:, s2 * 256:(s2 + 1) * 256])

        # ================= phase 3: banded conv per half =================
        def emit_ph3(h):
            q_ap = qm_r[h][:, 0:K * LQ]
            m_ap = qm_r[h][:, K * LQ:2 * K * LQ]
            hm_all = cfp.tile([128, ND * K * LQ], f16, tag="hm",
                              name=f"hm{h}")
            r2t = cfp.tile([128, K * LQ], f16, tag="r2t", name="r2t")
            for i in range(ND):
                d = D_LO + i
                sl = hm_all[:, i * K * LQ:(i + 1) * K * LQ]
                nc.vector.tensor_scalar_add(out=sl, in0=q_ap,
                                            scalar1=float(1 - d))
                nc.vector.tensor_scalar(out=r2t, in0=q_ap,
                                        scalar1=float(d + 1), scalar2=-1.0,
                                        op0=OP.subtract, op1=OP.mult)
                nc.vector.tensor_tensor(out=sl, in0=sl, in1=r2t, op=OP.min)
                nc.vector.tensor_scalar_max(out=sl, in0=sl, scalar1=0.0)
            m_rep = bass.AP(tensor=m_ap.tensor, offset=m_ap.offset,
                            ap=[m_ap.ap[0], [0, ND], [1, K * LQ]])
            nc.vector.tensor_tensor(
                out=hm_all[:].rearrange("p (d f) -> p d f", d=ND),
                in0=hm_all[:].rearrange("p (d f) -> p d f", d=ND),
                in1=m_rep, op=OP.mult)
            cd_all = cfp.tile([128, ND * LQ], f16, tag="cd", name=f"cd{h}")
            hm_v = hm_all[:].rearrange("p (d k f) -> p d k f", d=ND, k=K)
            cd_v = cd_all[:].rearrange("p (d f) -> p d f", d=ND)
            nc.vector.tensor_tensor(out=cd_v, in0=hm_v[:, :, 0, :],
                                    in1=hm_v[:, :, 1, :], op=OP.add)
            nc.vector.tensor_tensor(out=cd_v, in0=cd_v,
                                    in1=hm_v[:, :, 2, :], op=OP.add)

            xa_r_v = xa_r[h][:].rearrange("p (c wn) -> p c wn", c=GC)
            out_t = out_d[:]
            ost_all = outp.tile([128, GC * LQ], f16, tag="ost_all",
                                name=f"ost_all{h}")
            for cp in range(NCP):
                tmps = []
                for i in range(ND):
                    d = D_LO + i
                    tmp = tmpp.tile([128, CPG * LQ], f16, tag=f"tmp{i}",
                                    name=f"tmp{i}")
                    cdi = cd_all[:, i * LQ:(i + 1) * LQ]
                    cb = bass.AP(tensor=cdi.tensor, offset=cdi.offset,
                                 ap=[cdi.ap[0], [0, CPG], [1, LQ]])
                    # Pool is idle in the tail; let it take a slice of the
                    # last half's products to shorten the DVE-bound tail.
                    peng = (nc.gpsimd if (cp >= 5 + 2 * (1 - h) and i == 4)
                            else nc.vector)
                    peng.tensor_tensor(
                        out=tmp[:].rearrange("p (c f) -> p c f", c=CPG),
                        in0=xa_r_v[:, cp * CPG:(cp + 1) * CPG,
                                   HALO + d:HALO + d + LQ],
                        in1=cb, op=OP.mult)
                    tmps.append(tmp)
                o0 = cp * CPG * LQ
                if cp in DSUM_DVE_CP:
                    nc.vector.tensor_tensor(out=tmps[0], in0=tmps[0],
                                            in1=tmps[1], op=OP.add)
                    nc.vector.tensor_tensor(out=tmps[2], in0=tmps[2],
                                            in1=tmps[3], op=OP.add)
                    nc.vector.tensor_tensor(out=tmps[0], in0=tmps[0],
                                            in1=tmps[2], op=OP.add)
                    nc.vector.tensor_tensor(
                        out=ost_all[:, o0:o0 + CPG * LQ], in0=tmps[0],
                        in1=tmps[4], op=OP.add)
                else:
                    for jp in range(CPG // 2):
                        acc = psum_a.tile([128, 2 * LQ], f32, tag="acc",
                                          name="acc")
                        for jj in range(2):
                            j = jp * 2 + jj
                            for n_i in range(ND):
                                nc.tensor.matmul(
                                    acc[:, jj * LQ:(jj + 1) * LQ], id16,
                                    tmps[n_i][:, j * LQ:(j + 1) * LQ],
                                    start=(n_i == 0), stop=(n_i == ND - 1))
                        oj = o0 + jp * 2 * LQ
                        if (cp, jp) in OST_POOL_JP:
                            nc.gpsimd.tensor_copy(
                                out=ost_all[:, oj:oj + 2 * LQ], in_=acc)
                        else:
                            nc.scalar.copy(
                                out=ost_all[:, oj:oj + 2 * LQ], in_=acc)
            for g in range(G):
                hw_dma(
                    out=bass.AP(
                        tensor=out_t.tensor,
                        offset=out_t.offset + g * GC * L + h * LH,
                        ap=[[CH, NCH_H], [L, GC], [1, LQ]]),
                    in_=ost_all[g * 16:g * 16 + 16, :]
                    .rearrange("s (c f) -> s c f", c=GC))

        # driver: front-ends first (conv overlaps them via per-engine
        # in-order streams)
        emit_ph2a(0)
        emit_ph2a(1)
        emit_ph2b(0)
        emit_ph2b(1)
        emit_ph3(0)
        emit_ph3(1)

    return nc


def _prep_params(ln_w, ln_b, om_w, om_b):
    cst32 = np.zeros((128, 6), np.float32)
    cst32[:, 0] = ln_w[0:128]
    cst32[:, 1] = ln_w[128:256]
    cst32[:, 2] = ln_b[0:128]
    cst32[:, 3] = ln_b[128:256]
    cst32[:, 4] = 1.0
    # bias48 in (g, qm, k) order with conv grid fold for qm=0 (offsets)
    for g in range(G):
        for k in range(K):
            cst32[g * 6 + k, 5] = om_b[g * K + k] + (k - 1.0)
            cst32[g * 6 + 3 + k, 5] = om_b[G * K + g * K + k]
    cst16 = np.zeros((128, 226), np.float16)
    omwT = om_w.T.astype(np.float16)          # [C, 48] original row order
    perm = np.zeros(48, np.int64)
    for g in range(G):
        for k in range(K):
            perm[g * 6 + k] = g * K + k
            perm[g * 6 + 3 + k] = G * K + g * K + k
    omwTp = omwT[:, perm]                      # columns in (g,qm,k) order
    cst16[:, 0:48] = omwTp[0:128]
    cst16[:, 48:96] = omwTp[128:256]
    cst16[:, 96:224] = np.eye(128, dtype=np.float16)
    cst16[:, 224] = 1.0
    return {"cst32": cst32, "cst16": cst16}


def kernel(x, ln_w, ln_b, om_w, om_b):
    _install_patch()
    from concourse.bass_utils import run_bass_kernel_spmd

    if "nc" not in _cache:
        _cache["nc"] = _build_module()
    nc = _cache["nc"]

    x = np.ascontiguousarray(np.asarray(x, dtype=np.float32))
    params = _prep_params(np.asarray(ln_w, np.float32),
                          np.asarray(ln_b, np.float32),
                          np.asarray(om_w, np.float32),
                          np.asarray(om_b, np.float32))
    in_maps = [dict(params, x=x[n]) for n in range(N)]
    res = run_bass_kernel_spmd(nc, in_maps, core_ids=list(range(N)))
    return np.stack([res.results[n]["out"] for n in range(N)],
                    axis=0).astype(np.float32)


def _prep_inputs(inputs):
    x = np.ascontiguousarray(np.asarray(inputs["x"], dtype=np.float32))
    params = _prep_params(np.asarray(inputs["ln_w"], np.float32),
                          np.asarray(inputs["ln_b"], np.float32),
                          np.asarray(inputs["om_w"], np.float32),
                          np.asarray(inputs["om_b"], np.float32))
    return [dict(params, x=x[n]) for n in range(N)]


def run_traced(inputs):
    _install_patch()
    from concourse.bass_utils import run_bass_kernel_spmd
    if "nc" not in _cache:
        _cache["nc"] = _build_module()
    return run_bass_kernel_spmd(_cache["nc"], _prep_inputs(inputs),
                                core_ids=list(range(N)), trace=True)


# revision 63
# speedup vs baseline: 1.0865x; 1.0121x over previous
"""DCNv4-1D fused Trainium2 kernel v2. Data-parallel over batch N, 8 cores.

Per core (one sample), redesigned against the TimelineSim cost model:
  - x loaded via SWDGE cast-DMA f32->f16 (cost model charges OUT bytes).
  - LN stats: f16 ones-matmuls on PE into PSUM [33,512] (sum row 0, sumsq
    row 32); ACT copies to an f16 row; tiny DMAs repack to [128,w]; DVE/ACT
    finish rs/murs; DRAM round-trip broadcasts rs|murs to [128, 2*SC] f16.
    All Sqrt before any Gelu (2 ACT table loads total).
  - Apply: v = x16*rs_b, u = v - murs_b (DVE f16 TT, 2x mode), then
    xa = gelu(lnw*u + lnb) on ACT into xa_full with zeroed halo guards.
  - om = om_wT' @ xa on PE (f16), ACT adds bias(+grid fold), scatter DMAs
    land rows in qm_r [(g,s16), (o,f)] layout.
  - Banded conv (5 diagonals d=-2..2): coeff on DVE (tensor_scalar 4x via
    abs_max trick + TT mask/ksum); products coeff*xa on DVE f16 TT;
    d-sum mostly via PE id16 matmuls into PSUM + ACT copy-out, partly as
    DVE TT adds writing f16 directly (knob).
  - L split in 2 halves: half-0 conv overlaps half-1 front-end.
"""

import json

import numpy as np

N, C, L = 8, 256, 8192
G, K, GC = 8, 3, 32
LN_EPS = 1e-6
NCT = 2
NHALF = 2
LH = L // NHALF          # 4096
SC = 2048                # stats superchunk
NSC = L // SC            # 4 (2 per half)
SW = SC // 128           # 16 stats cols per partition per sc
CH = 256                 # conv chunk (16 per half -> partitions (g, s))
NCH_H = LH // CH         # 16
LQ = CH
HALO = 4
WIN = CH + 2 * HALO      # 264
D_LO, D_HI = -2, 2
ND = D_HI - D_LO + 1     # 5
CPG = 4                  # channels per product op
NCP = GC // CPG          # 4 product blocks per half

# ---- tuning knobs ----
DSUM_DVE_CP = ()         # cp blocks whose d-sum runs on DVE (rest PE+ACT)
OST_POOL_JP = ()         # (cp, jp) pairs whose PSUM->f16 copy runs on Pool
SCAT_SWDGE = True        # half the qm scatters on SWDGE (Pool) vs HWDGE

_cache = {}
_DBG_PSUM_INIT = False
_DBG_RELU = False


# --- BIR post-pass: this walrus build rejects >1 sync wait per instruction;
# split extras onto same-engine NoOps inserted just before the owner. ---
def _split_multi_waits(bir_json: bytes, max_waits: int = 1) -> bytes:
    j = json.loads(bir_json)
    n = [0]

    def fresh():
        n[0] += 1
        return f"I-wsplit-{n[0]}"

    for fn in j.get("functions", []):
        for bb in fn.get("basicblocks", []) or fn.get("blocks", []) or []:
            out = []
            for inst in bb.get("instructions", []):
                si = inst.get("sync_info")
                waits = (si or {}).get("on_wait") or []
                if len(waits) > max_waits:
                    for w in waits[:-max_waits]:
                        out.append({
                            "debug": inst.get("debug", 0),
                            "engine": inst["engine"],
                            "ins": [], "outs": [],
                            "name": fresh(),
                            "opcode": "NoOp",
                            "sync_info": {"on_update": [], "on_wait": [w]},
                        })
                    si["on_wait"] = waits[-max_waits:]
                out.append(inst)
            bb["instructions"] = out
    return json.dumps(j).encode()


def _install_patch():
    import os

    # The elide-DMA sem optimization assumes per-ring FIFO but DMAHW lanes
    # are shared across the SP/ACT rings; with our cross-ring interleave it
    # miscounts and consumers fire early. Disable it.
    os.environ["BACC_ELIDE_DMA_OPT_LIMIT"] = "0"
    import concourse.tile_sem_assignment as _tsa
    _tsa._opt_limit = 0

    import concourse.bass2jax as bass2jax
    import concourse.bass_utils as bass_utils

    if getattr(bass2jax.compile_bir_kernel, "_wsplit", False):
        return
    orig = bass_utils.compile_bir_kernel

    def patched(bir_json, tmpdir, neff_name="file.neff"):
        return orig(_split_multi_waits(bir_json), tmpdir, neff_name=neff_name)

    patched._wsplit = True
    bass_utils.compile_bir_kernel = patched
    bass2jax.compile_bir_kernel = patched


def _build_module():
    import contextlib

    import concourse.bass as bass
    import concourse.tile as tile
    from concourse import mybir

    f32 = mybir.dt.float32
    f16 = mybir.dt.float16
    AF = mybir.ActivationFunctionType
    GELU = AF.Relu if _DBG_RELU else AF.Gelu
    OP = mybir.AluOpType

    nc = bass.Bass()

    x_d = nc.dram_tensor("x", [C, L], f32, kind="ExternalInput")
    cst32_d = nc.dram_tensor("cst32", [128, 6], f32, kind="ExternalInput")
    cst16_d = nc.dram_tensor("cst16", [128, 226], f16, kind="ExternalInput")
    scr_d = nc.dram_tensor("scr", [NSC, 2 * SC], f16, kind="Internal")
    out_d = nc.dram_tensor("out", [C, L], f16, kind="ExternalOutput")

    with tile.TileContext(nc) as tc, contextlib.ExitStack() as ctx:
        const = ctx.enter_context(tc.tile_pool(name="const", bufs=1))
        persist = ctx.enter_context(tc.tile_pool(name="persist", bufs=1))
        xqp = ctx.enter_context(tc.tile_pool(name="xq", bufs=1))
        srp = ctx.enter_context(tc.tile_pool(name="srp", bufs=2))
        stp = ctx.enter_context(tc.tile_pool(name="stp", bufs=2))
        rbp = ctx.enter_context(tc.tile_pool(name="rbp", bufs=3))
        vup = ctx.enter_context(tc.tile_pool(name="vup", bufs=2))
        omp = ctx.enter_context(tc.tile_pool(name="omp", bufs=2))
        xrp = ctx.enter_context(tc.tile_pool(name="xrp", bufs=2))
        qmp = ctx.enter_context(tc.tile_pool(name="qmp", bufs=2))
        cfp = ctx.enter_context(tc.tile_pool(name="cfp", bufs=1))
        tmpp = ctx.enter_context(tc.tile_pool(name="tmpp", bufs=2))
        outp = ctx.enter_context(tc.tile_pool(name="outp", bufs=1))
        psum = ctx.enter_context(tc.tile_pool(name="psum", bufs=2,
                                              space="PSUM"))
        psum_o = ctx.enter_context(tc.tile_pool(name="psum_o", bufs=2,
                                                space="PSUM"))
        psum_a = ctx.enter_context(tc.tile_pool(name="psum_a", bufs=4,
                                                space="PSUM"))

        # HWDGE lane purity: DMAHW lanes are assigned round-robin over ALL
        # hwdge DMAs; threshold waits are only sound if each lane is fed by
        # one FIFO ring. Strict SP/ACT alternation keeps even lanes SP-only
        # and odd lanes ACT-only (NUM_HWDGE_SEMS == 8).
        _hwflip = [0]

        def hw_dma(out, in_):
            eng = nc.sync if _hwflip[0] % 2 == 0 else nc.scalar
            _hwflip[0] += 1
            eng.dma_start(out=out, in_=in_)

        # ---------------- constants ----------------
        cst32 = const.tile([128, 6], f32, tag="cst32", name="cst32")
        hw_dma(out=cst32, in_=cst32_d[:])
        cst16 = const.tile([128, 226], f16, tag="cst16", name="cst16")
        hw_dma(out=cst16, in_=cst16_d[:])
        lnw_c = [cst32[:, ct:ct + 1] for ct in range(NCT)]
        lnb_c = [cst32[:, 2 + ct:3 + ct] for ct in range(NCT)]
        bias48 = cst32[0:48, 5:6]
        omwT = [cst16[:, ct * 48:(ct + 1) * 48] for ct in range(NCT)]
        id16 = cst16[:, 96:224]
        onesc16 = cst16[:, 224:225]
        eps_c = const.tile([128, 1], f32, tag="eps", name="eps")
        nc.vector.memset(eps_c, LN_EPS)

        # ---------------- persistent tensors ----------------
        x16 = [persist.tile([128, L], f16, tag=f"x16_{ct}",
                            name=f"x16_{ct}") for ct in range(NCT)]
        xa_full = [persist.tile([128, 2 * HALO + L], f16, tag=f"xaf{ct}",
                                name=f"xaf{ct}") for ct in range(NCT)]
        for ct in range(NCT):
            nc.vector.memset(xa_full[ct][:, 0:HALO], 0.0)
            nc.vector.memset(xa_full[ct][:, HALO + L:2 * HALO + L], 0.0)

        # x cast loads (SWDGE), chunked per superchunk so stats start early
        for sc in range(NSC):
            for ct in range(NCT):
                nc.gpsimd.dma_start(
                    out=x16[ct][:, sc * SC:(sc + 1) * SC],
                    in_=x_d[ct * 128:(ct + 1) * 128, sc * SC:(sc + 1) * SC])

        rb_l = [None] * NSC

        # ============ phase 1: LN stats, staged across superchunks ============
        # (same-stage DMAs are adjacent in each queue so the four superchunk
        # round-trips pipeline instead of serializing end-to-end)
        spk_l, qpk_l, rsmu_l = [], [], []
        for sc in range(NSC):
            lo = sc * SC
            srow = srp.tile([33, SC], f16, tag="srow", name="srow")
            xq = [None] * NCT
            for ct in range(NCT):
                xq[ct] = xqp.tile([128, SC], f16, tag=f"xq{ct}",
                                  name=f"xq{ct}")
                nc.vector.tensor_tensor(out=xq[ct],
                                        in0=x16[ct][:, lo:lo + SC],
                                        in1=x16[ct][:, lo:lo + SC],
                                        op=OP.mult)
            for c in range(SC // 512):
                cf = c * 512
                st_ps = psum.tile([33, 512], f32, tag="st", name="st")
                if _DBG_PSUM_INIT:
                    nc.vector.memset(st_ps[1:32, :], 0.0)
                for ct in range(NCT):
                    nc.tensor.matmul(st_ps[0:1, :], onesc16,
                                     x16[ct][:, lo + cf:lo + cf + 512],
                                     start=(ct == 0), stop=(ct == NCT - 1))
                for ct in range(NCT):
                    nc.tensor.matmul(st_ps[32:33, :], onesc16,
                                     xq[ct][:, cf:cf + 512],
                                     start=(ct == 0), stop=(ct == NCT - 1))
                nc.scalar.copy(out=srow[:, cf:cf + 512], in_=st_ps)

            spk = stp.tile([128, SW], f16, tag=f"spk{sc}", name=f"spk{sc}")
            hw_dma(
                out=spk,
                in_=srow[0:1, :].rearrange("one (p w) -> one p w", p=128))
            qpk = stp.tile([128, SW], f16, tag=f"qpk{sc}", name=f"qpk{sc}")
            hw_dma(
                out=qpk,
                in_=srow[32:33, :].rearrange("one (p w) -> one p w", p=128))
            spk_l.append(spk)
            qpk_l.append(qpk)

        for sc in range(NSC):
            spk, qpk = spk_l[sc], qpk_l[sc]
            mu_t = stp.tile([128, SW], f32, tag="mu_t", name="mu_t")
            nc.vector.tensor_scalar_mul(out=mu_t, in0=spk, scalar1=1.0 / C)
            musq = stp.tile([128, SW], f32, tag="musq", name="musq")
            nc.vector.tensor_tensor(out=musq, in0=mu_t, in1=mu_t,
                                    op=OP.mult)
            varq = stp.tile([128, SW], f32, tag="varq", name="varq")
            nc.vector.scalar_tensor_tensor(out=varq, in0=qpk, scalar=1.0 / C,
                                           in1=musq, op0=OP.mult,
                                           op1=OP.subtract)
            sd = stp.tile([128, SW], f32, tag="sd", name="sd")
            nc.scalar.activation(out=sd, in_=varq, func=AF.Sqrt,
                                 bias=eps_c, scale=1.0)
            rs32 = stp.tile([128, SW], f32, tag="rs32", name="rs32")
            nc.vector.reciprocal(out=rs32, in_=sd)
            rsmu = stp.tile([128, 2 * SW], f16, tag=f"rsmu{sc}",
                            name=f"rsmu{sc}")
            nc.vector.tensor_scalar_mul(out=rsmu[:, 0:SW], in0=rs32,
                                        scalar1=1.0)
            nc.vector.scalar_tensor_tensor(out=rsmu[:, SW:2 * SW], in0=spk,
                                           scalar=1.0 / C, in1=rs32,
                                           op0=OP.mult, op1=OP.mult)
            rsmu_l.append(rsmu)
            nc.gpsimd.dma_start(
                out=scr_d[sc:sc + 1, :].rearrange("one (p w) -> one p w",
                                                  p=128),
                in_=rsmu)

        for sc in range(NSC):
            halves = []
            row = scr_d[sc:sc + 1, :]
            for rh in range(2):
                rb = rbp.tile([128, SC], f16, tag=f"rb{rh}", name="rb")
                hw_dma(
                    out=rb,
                    in_=bass.AP(tensor=row.tensor,
                                offset=row.offset + rh * SC,
                                ap=[[0, 128], [1, SC]]))
                halves.append(rb)
            rb_l[sc] = halves

        # per-half state
        qm_r = [None] * NHALF
        xa_r = [None] * NHALF
        prev_hi = {0: -1, 1: -1}

        def windows_dma(h, s, ct, dma):
            # xa window for conv chunk s of half h, channel tile ct
            st = h * LH + s * CH
            dma(
                out=xa_r[h][ct * 64 + s:ct * 64 + s + 49:16, :]
                .rearrange("g (cc w) -> g cc w", cc=GC),
                in_=xa_full[ct][:, st:st + WIN])

        def flush_windows(h, s_hi):
            # inline: only SWDGE windows (HWDGE-ring ones would stall the
            # issuing engine's SEQ between gelus); HWDGE half is deferred
            # to emit_ph2b once the half's gelus are all emitted.
            s_hi = min(s_hi, NCH_H - 1)
            for s in range(prev_hi[h] + 1, s_hi + 1):
                for ct in range(NCT):
                    if (s + ct) % 2 == 1:
                        windows_dma(h, s, ct, nc.gpsimd.dma_start)
            prev_hi[h] = max(prev_hi[h], s_hi)

        def flush_windows_hw(h):
            for s in range(NCH_H):
                for ct in range(NCT):
                    if (s + ct) % 2 == 0:
                        windows_dma(h, s, ct, hw_dma)

        # ============= phase 2: apply + gelu + om (per half) =============
        APW = 1024  # apply op width
        SC_PER_H = LH // SC  # 2

        def emit_apply(sc, lo_off, width):
            # LN apply + gelu for x columns [sc*SC+lo_off, +width)
            rh = (lo_off // SW) // 64
            rb_v = rb_l[sc][rh][:].rearrange("q (p half w) -> q p half w",
                                             p=64, half=2)
            rs_sl = rb_v[:, :, 0, :]
            mu_sl = rb_v[:, :, 1, :]
            lo = sc * SC + lo_off
            pw = width // SW
            p0 = (lo_off // SW) % 64
            for ct in range(NCT):
                xs = x16[ct][:, lo:lo + width].rearrange(
                    "p (a w) -> p a w", a=pw)
                v = vup.tile([128, APW], f16, tag=f"v{ct}", name=f"v{ct}")
                vv = v[:, 0:width].rearrange("p (a w) -> p a w", a=pw)
                nc.vector.tensor_tensor(
                    out=vv, in0=xs, in1=rs_sl[:, p0:p0 + pw, :], op=OP.mult)
                nc.vector.tensor_tensor(
                    out=vv, in0=vv, in1=mu_sl[:, p0:p0 + pw, :],
                    op=OP.subtract)
                nc.scalar.activation(
                    out=xa_full[ct][:, HALO + lo:HALO + lo + width],
                    in_=v[:, 0:width], func=GELU, bias=lnb_c[ct],
                    scale=lnw_c[ct])

        def emit_ph2a(h):
            xa_r[h] = xrp.tile([128, GC * WIN], f16, tag="xa_r",
                               name=f"xa_r{h}")
            for sc_h in range(SC_PER_H):
                sc = h * SC_PER_H + sc_h
                for a2 in range(SC // APW):
                    emit_apply(sc, a2 * APW, APW)
                    cov = sc_h * SC + (a2 + 1) * APW
                    flush_windows(h, (cov - CH - HALO) // CH)
            if h == 0:
                # pre-compute h1's first 512 cols so h0's last window (and
                # thus h0's conv) doesn't wait for h1's front-end. h1
                # recomputes the same values later (idempotent).
                emit_apply(SC_PER_H, 0, 512)
                flush_windows(h, NCH_H - 1)

        def emit_ph2b(h):
            qm_r[h] = qmp.tile([128, 2 * K * LQ], f16, tag="qm_r",
                               name=f"qm_r{h}")
            if h == NHALF - 1:
                flush_windows(h, NCH_H - 1)
            flush_windows_hw(h)
            for gp in range(LH // 512):
                lo5 = h * LH + gp * 512
                om_ps = psum_o.tile([48, 512], f32, tag="om", name="om")
                for ct in range(NCT):
                    nc.tensor.matmul(
                        om_ps, omwT[ct],
                        xa_full[ct][:, HALO + lo5:HALO + lo5 + 512],
                        start=(ct == 0), stop=(ct == NCT - 1))
                om_st = omp.tile([48, 512], f16, tag="om_st", name="om_st")
                if h == 0:
                    # DVE is stalled waiting these scatters anyway; doing the
                    # bias+convert there skips the ACT queue (which is still
                    # busy with h1's gelus) and feeds coeff(h0) ~10us earlier
                    nc.vector.tensor_scalar_add(out=om_st, in0=om_ps,
                                                scalar1=bias48)
                else:
                    nc.scalar.activation(out=om_st, in_=om_ps,
                                         func=AF.Identity, bias=bias48,
                                         scale=1.0)
                for s2 in range(2):
                    s = gp * 2 + s2  # conv chunk within half
                    hw_dma(
                        out=qm_r[h][s:128:16, :]
                        .rearrange("g (o f) -> g o f", o=6),
                        in_=om_st[:, s2 * 256:(s2 + 1) * 256])

        # ================= phase 3: banded conv per half =================
        def emit_ph3(h):
            q_ap = qm_r[h][:, 0:K * LQ]
            m_ap = qm_r[h][:, K * LQ:2 * K * LQ]
            hm_all = cfp.tile([128, ND * K * LQ], f16, tag="hm",
                              name=f"hm{h}")
            r2t = cfp.tile([128, K * LQ], f16, tag="r2t", name="r2t")
            for i in range(ND):
                d = D_LO + i
                sl = hm_all[:, i * K * LQ:(i + 1) * K * LQ]
                nc.vector.tensor_scalar_add(out=sl, in0=q_ap,
                                            scalar1=float(1 - d))
                nc.vector.tensor_scalar(out=r2t, in0=q_ap,
                                        scalar1=float(d + 1), scalar2=-1.0,
                                        op0=OP.subtract, op1=OP.mult)
                nc.vector.tensor_tensor(out=sl, in0=sl, in1=r2t, op=OP.min)
                nc.vector.tensor_scalar_max(out=sl, in0=sl, scalar1=0.0)
            m_rep = bass.AP(tensor=m_ap.tensor, offset=m_ap.offset,
                            ap=[m_ap.ap[0], [0, ND], [1, K * LQ]])
            nc.vector.tensor_tensor(
                out=hm_all[:].rearrange("p (d f) -> p d f", d=ND),
                in0=hm_all[:].rearrange("p (d f) -> p d f", d=ND),
                in1=m_rep, op=OP.mult)
            cd_all = cfp.tile([128, ND * LQ], f16, tag="cd", name=f"cd{h}")
            hm_v = hm_all[:].rearrange("p (d k f) -> p d k f", d=ND, k=K)
            cd_v = cd_all[:].rearrange("p (d f) -> p d f", d=ND)
            nc.vector.tensor_tensor(out=cd_v, in0=hm_v[:, :, 0, :],
                                    in1=hm_v[:, :, 1, :], op=OP.add)
            nc.vector.tensor_tensor(out=cd_v, in0=cd_v,
                                    in1=hm_v[:, :, 2, :], op=OP.add)

            xa_r_v = xa_r[h][:].rearrange("p (c wn) -> p c wn", c=GC)
            out_t = out_d[:]
            ost_all = outp.tile([128, GC * LQ], f16, tag="ost_all",
                                name=f"ost_all{h}")
            for cp in range(NCP):
                tmps = []
                for i in range(ND):
                    d = D_LO + i
                    tmp = tmpp.tile([128, CPG * LQ], f16, tag=f"tmp{i}",
                                    name=f"tmp{i}")
                    cdi = cd_all[:, i * LQ:(i + 1) * LQ]
                    cb = bass.AP(tensor=cdi.tensor, offset=cdi.offset,
                                 ap=[cdi.ap[0], [0, CPG], [1, LQ]])
                    # Pool is idle in the tail; let it take a slice of the
                    # last half's products to shorten the DVE-bound tail.
                    peng = (nc.gpsimd if (cp >= 2 + 5 * (1 - h) and i == 4)
                            else nc.vector)
                    peng.tensor_tensor(
                        out=tmp[:].rearrange("p (c f) -> p c f", c=CPG),
                        in0=xa_r_v[:, cp * CPG:(cp + 1) * CPG,
                                   HALO + d:HALO + d + LQ],
                        in1=cb, op=OP.mult)
                    tmps.append(tmp)
                o0 = cp * CPG * LQ
                if cp in DSUM_DVE_CP:
                    nc.vector.tensor_tensor(out=tmps[0], in0=tmps[0],
                                            in1=tmps[1], op=OP.add)
                    nc.vector.tensor_tensor(out=tmps[2], in0=tmps[2],
                                            in1=tmps[3], op=OP.add)
                    nc.vector.tensor_tensor(out=tmps[0], in0=tmps[0],
                                            in1=tmps[2], op=OP.add)
                    nc.vector.tensor_tensor(
                        out=ost_all[:, o0:o0 + CPG * LQ], in0=tmps[0],
                        in1=tmps[4], op=OP.add)
                else:
                    for jp in range(CPG // 2):
                        acc = psum_a.tile([128, 2 * LQ], f32, tag="acc",
                                          name="acc")
                        for jj in range(2):
                            j = jp * 2 + jj
                            for n_i in range(ND):
                                nc.tensor.matmul(
                                    acc[:, jj * LQ:(jj + 1) * LQ], id16,
                                    tmps[n_i][:, j * LQ:(j + 1) * LQ],
                                    start=(n_i == 0), stop=(n_i == ND - 1))
                        oj = o0 + jp * 2 * LQ
                        if (cp, jp) in OST_POOL_JP:
                            nc.gpsimd.tensor_copy(
                                out=ost_all[:, oj:oj + 2 * LQ], in_=acc)
                        else:
                            nc.scalar.copy(
                                out=ost_all[:, oj:oj + 2 * LQ], in_=acc)
            for g in range(G):
                hw_dma(
                    out=bass.AP(
                        tensor=out_t.tensor,
                        offset=out_t.offset + g * GC * L + h * LH,
                        ap=[[CH, NCH_H], [L, GC], [1, LQ]]),
                    in_=ost_all[g * 16:g * 16 + 16, :]
                    .rearrange("s (c f) -> s c f", c=GC))

        # driver: front-ends first (conv overlaps them via per-engine
        # in-order streams)
        emit_ph2a(0)
        emit_ph2a(1)
        emit_ph2b(0)
        emit_ph2b(1)
        emit_ph3(0)
        emit_ph3(1)

    return nc


def _prep_params(ln_w, ln_b, om_w, om_b):
    cst32 = np.zeros((128, 6), np.float32)
    cst32[:, 0] = ln_w[0:128]
    cst32[:, 1] = ln_w[128:256]
    cst32[:, 2] = ln_b[0:128]
    cst32[:, 3] = ln_b[128:256]
    cst32[:, 4] = 1.0
    # bias48 in (g, qm, k) order with conv grid fold for qm=0 (offsets)
    for g in range(G):
        for k in range(K):
            cst32[g * 6 + k, 5] = om_b[g * K + k] + (k - 1.0)
            cst32[g * 6 + 3 + k, 5] = om_b[G * K + g * K + k]
    cst16 = np.zeros((128, 226), np.float16)
    omwT = om_w.T.astype(np.float16)          # [C, 48] original row order
    perm = np.zeros(48, np.int64)
    for g in range(G):
        for k in range(K):
            perm[g * 6 + k] = g * K + k
            perm[g * 6 + 3 + k] = G * K + g * K + k
    omwTp = omwT[:, perm]                      # columns in (g,qm,k) order
    cst16[:, 0:48] = omwTp[0:128]
    cst16[:, 48:96] = omwTp[128:256]
    cst16[:, 96:224] = np.eye(128, dtype=np.float16)
    cst16[:, 224] = 1.0
    return {"cst32": cst32, "cst16": cst16}


def kernel(x, ln_w, ln_b, om_w, om_b):
    _install_patch()
    from concourse.bass_utils import run_bass_kernel_spmd

    if "nc" not in _cache:
        _cache["nc"] = _build_module()
    nc = _cache["nc"]

    x = np.ascontiguousarray(np.asarray(x, dtype=np.float32))
    params = _prep_params(np.asarray(ln_w, np.float32),
                          np.asarray(ln_b, np.float32),
                          np.asarray(om_w, np.float32),
                          np.asarray(om_b, np.float32))
    in_maps = [dict(params, x=x[n]) for n in range(N)]
    res = run_bass_kernel_spmd(nc, in_maps, core_ids=list(range(N)))
    return np.stack([res.results[n]["out"] for n in range(N)],
                    axis=0).astype(np.float32)


def _prep_inputs(inputs):
    x = np.ascontiguousarray(np.asarray(inputs["x"], dtype=np.float32))
    params = _prep_params(np.asarray(inputs["ln_w"], np.float32),
                          np.asarray(inputs["ln_b"], np.float32),
                          np.asarray(inputs["om_w"], np.float32),
                          np.asarray(inputs["om_b"], np.float32))
    return [dict(params, x=x[n]) for n in range(N)]


def run_traced(inputs):
    _install_patch()
    from concourse.bass_utils import run_bass_kernel_spmd
    if "nc" not in _cache:
        _cache["nc"] = _build_module()
    return run_bass_kernel_spmd(_cache["nc"], _prep_inputs(inputs),
                                core_ids=list(range(N)), trace=True)


# revision 65
# speedup vs baseline: 1.0897x; 1.0029x over previous
"""DCNv4-1D fused Trainium2 kernel v2. Data-parallel over batch N, 8 cores.

Per core (one sample), redesigned against the TimelineSim cost model:
  - x loaded via SWDGE cast-DMA f32->f16 (cost model charges OUT bytes).
  - LN stats: f16 ones-matmuls on PE into PSUM [33,512] (sum row 0, sumsq
    row 32); ACT copies to an f16 row; tiny DMAs repack to [128,w]; DVE/ACT
    finish rs/murs; DRAM round-trip broadcasts rs|murs to [128, 2*SC] f16.
    All Sqrt before any Gelu (2 ACT table loads total).
  - Apply: v = x16*rs_b, u = v - murs_b (DVE f16 TT, 2x mode), then
    xa = gelu(lnw*u + lnb) on ACT into xa_full with zeroed halo guards.
  - om = om_wT' @ xa on PE (f16), ACT adds bias(+grid fold), scatter DMAs
    land rows in qm_r [(g,s16), (o,f)] layout.
  - Banded conv (5 diagonals d=-2..2): coeff on DVE (tensor_scalar 4x via
    abs_max trick + TT mask/ksum); products coeff*xa on DVE f16 TT;
    d-sum mostly via PE id16 matmuls into PSUM + ACT copy-out, partly as
    DVE TT adds writing f16 directly (knob).
  - L split in 2 halves: half-0 conv overlaps half-1 front-end.
"""

import json

import numpy as np

N, C, L = 8, 256, 8192
G, K, GC = 8, 3, 32
LN_EPS = 1e-6
NCT = 2
NHALF = 2
LH = L // NHALF          # 4096
SC = 2048                # stats superchunk
NSC = L // SC            # 4 (2 per half)
SW = SC // 128           # 16 stats cols per partition per sc
CH = 256                 # conv chunk (16 per half -> partitions (g, s))
NCH_H = LH // CH         # 16
LQ = CH
HALO = 4
WIN = CH + 2 * HALO      # 264
D_LO, D_HI = -2, 2
ND = D_HI - D_LO + 1     # 5
CPG = 4                  # channels per product op
NCP = GC // CPG          # 4 product blocks per half

# ---- tuning knobs ----
DSUM_DVE_CP = ()         # cp blocks whose d-sum runs on DVE (rest PE+ACT)
OST_POOL_JP = ()         # (cp, jp) pairs whose PSUM->f16 copy runs on Pool
SCAT_SWDGE = True        # half the qm scatters on SWDGE (Pool) vs HWDGE

_cache = {}
_DBG_PSUM_INIT = False
_DBG_RELU = False


# --- BIR post-pass: this walrus build rejects >1 sync wait per instruction;
# split extras onto same-engine NoOps inserted just before the owner. ---
def _split_multi_waits(bir_json: bytes, max_waits: int = 1) -> bytes:
    j = json.loads(bir_json)
    n = [0]

    def fresh():
        n[0] += 1
        return f"I-wsplit-{n[0]}"

    for fn in j.get("functions", []):
        for bb in fn.get("basicblocks", []) or fn.get("blocks", []) or []:
            out = []
            for inst in bb.get("instructions", []):
                si = inst.get("sync_info")
                waits = (si or {}).get("on_wait") or []
                if len(waits) > max_waits:
                    for w in waits[:-max_waits]:
                        out.append({
                            "debug": inst.get("debug", 0),
                            "engine": inst["engine"],
                            "ins": [], "outs": [],
                            "name": fresh(),
                            "opcode": "NoOp",
                            "sync_info": {"on_update": [], "on_wait": [w]},
                        })
                    si["on_wait"] = waits[-max_waits:]
                out.append(inst)
            bb["instructions"] = out
    return json.dumps(j).encode()


def _install_patch():
    import os

    # The elide-DMA sem optimization assumes per-ring FIFO but DMAHW lanes
    # are shared across the SP/ACT rings; with our cross-ring interleave it
    # miscounts and consumers fire early. Disable it.
    os.environ["BACC_ELIDE_DMA_OPT_LIMIT"] = "0"
    import concourse.tile_sem_assignment as _tsa
    _tsa._opt_limit = 0

    import concourse.bass2jax as bass2jax
    import concourse.bass_utils as bass_utils

    if getattr(bass2jax.compile_bir_kernel, "_wsplit", False):
        return
    orig = bass_utils.compile_bir_kernel

    def patched(bir_json, tmpdir, neff_name="file.neff"):
        return orig(_split_multi_waits(bir_json), tmpdir, neff_name=neff_name)

    patched._wsplit = True
    bass_utils.compile_bir_kernel = patched
    bass2jax.compile_bir_kernel = patched


def _build_module():
    import contextlib

    import concourse.bass as bass
    import concourse.tile as tile
    from concourse import mybir

    f32 = mybir.dt.float32
    f16 = mybir.dt.float16
    AF = mybir.ActivationFunctionType
    GELU = AF.Relu if _DBG_RELU else AF.Gelu
    OP = mybir.AluOpType

    nc = bass.Bass()

    x_d = nc.dram_tensor("x", [C, L], f32, kind="ExternalInput")
    cst32_d = nc.dram_tensor("cst32", [128, 6], f32, kind="ExternalInput")
    cst16_d = nc.dram_tensor("cst16", [128, 226], f16, kind="ExternalInput")
    scr_d = nc.dram_tensor("scr", [NSC, 2 * SC], f16, kind="Internal")
    out_d = nc.dram_tensor("out", [C, L], f16, kind="ExternalOutput")

    with tile.TileContext(nc) as tc, contextlib.ExitStack() as ctx:
        const = ctx.enter_context(tc.tile_pool(name="const", bufs=1))
        persist = ctx.enter_context(tc.tile_pool(name="persist", bufs=1))
        xqp = ctx.enter_context(tc.tile_pool(name="xq", bufs=1))
        srp = ctx.enter_context(tc.tile_pool(name="srp", bufs=2))
        stp = ctx.enter_context(tc.tile_pool(name="stp", bufs=2))
        rbp = ctx.enter_context(tc.tile_pool(name="rbp", bufs=3))
        vup = ctx.enter_context(tc.tile_pool(name="vup", bufs=2))
        omp = ctx.enter_context(tc.tile_pool(name="omp", bufs=2))
        xrp = ctx.enter_context(tc.tile_pool(name="xrp", bufs=2))
        qmp = ctx.enter_context(tc.tile_pool(name="qmp", bufs=2))
        cfp = ctx.enter_context(tc.tile_pool(name="cfp", bufs=1))
        tmpp = ctx.enter_context(tc.tile_pool(name="tmpp", bufs=2))
        outp = ctx.enter_context(tc.tile_pool(name="outp", bufs=1))
        psum = ctx.enter_context(tc.tile_pool(name="psum", bufs=2,
                                              space="PSUM"))
        psum_o = ctx.enter_context(tc.tile_pool(name="psum_o", bufs=2,
                                                space="PSUM"))
        psum_a = ctx.enter_context(tc.tile_pool(name="psum_a", bufs=4,
                                                space="PSUM"))

        # HWDGE lane purity: DMAHW lanes are assigned round-robin over ALL
        # hwdge DMAs; threshold waits are only sound if each lane is fed by
        # one FIFO ring. Strict SP/ACT alternation keeps even lanes SP-only
        # and odd lanes ACT-only (NUM_HWDGE_SEMS == 8).
        _hwflip = [0]

        def hw_dma(out, in_):
            eng = nc.sync if _hwflip[0] % 2 == 0 else nc.scalar
            _hwflip[0] += 1
            eng.dma_start(out=out, in_=in_)

        # ---------------- constants ----------------
        cst32 = const.tile([128, 6], f32, tag="cst32", name="cst32")
        hw_dma(out=cst32, in_=cst32_d[:])
        cst16 = const.tile([128, 226], f16, tag="cst16", name="cst16")
        hw_dma(out=cst16, in_=cst16_d[:])
        lnw_c = [cst32[:, ct:ct + 1] for ct in range(NCT)]
        lnb_c = [cst32[:, 2 + ct:3 + ct] for ct in range(NCT)]
        bias48 = cst32[0:48, 5:6]
        omwT = [cst16[:, ct * 48:(ct + 1) * 48] for ct in range(NCT)]
        id16 = cst16[:, 96:224]
        onesc16 = cst16[:, 224:225]
        eps_c = const.tile([128, 1], f32, tag="eps", name="eps")
        nc.vector.memset(eps_c, LN_EPS)

        # ---------------- persistent tensors ----------------
        x16 = [persist.tile([128, L], f16, tag=f"x16_{ct}",
                            name=f"x16_{ct}") for ct in range(NCT)]
        xa_full = [persist.tile([128, 2 * HALO + L], f16, tag=f"xaf{ct}",
                                name=f"xaf{ct}") for ct in range(NCT)]
        for ct in range(NCT):
            nc.vector.memset(xa_full[ct][:, 0:HALO], 0.0)
            nc.vector.memset(xa_full[ct][:, HALO + L:2 * HALO + L], 0.0)

        # x cast loads (SWDGE), chunked per superchunk so stats start early
        for sc in range(NSC):
            for ct in range(NCT):
                nc.gpsimd.dma_start(
                    out=x16[ct][:, sc * SC:(sc + 1) * SC],
                    in_=x_d[ct * 128:(ct + 1) * 128, sc * SC:(sc + 1) * SC])

        rb_l = [None] * NSC

        # ============ phase 1: LN stats, staged across superchunks ============
        # (same-stage DMAs are adjacent in each queue so the four superchunk
        # round-trips pipeline instead of serializing end-to-end)
        spk_l, qpk_l, rsmu_l = [], [], []
        for sc in range(NSC):
            lo = sc * SC
            srow = srp.tile([33, SC], f16, tag="srow", name="srow")
            xq = [None] * NCT
            for ct in range(NCT):
                xq[ct] = xqp.tile([128, SC], f16, tag=f"xq{ct}",
                                  name=f"xq{ct}")
                nc.vector.tensor_tensor(out=xq[ct],
                                        in0=x16[ct][:, lo:lo + SC],
                                        in1=x16[ct][:, lo:lo + SC],
                                        op=OP.mult)
            for c in range(SC // 512):
                cf = c * 512
                st_ps = psum.tile([33, 512], f32, tag="st", name="st")
                if _DBG_PSUM_INIT:
                    nc.vector.memset(st_ps[1:32, :], 0.0)
                for ct in range(NCT):
                    nc.tensor.matmul(st_ps[0:1, :], onesc16,
                                     x16[ct][:, lo + cf:lo + cf + 512],
                                     start=(ct == 0), stop=(ct == NCT - 1))
                for ct in range(NCT):
                    nc.tensor.matmul(st_ps[32:33, :], onesc16,
                                     xq[ct][:, cf:cf + 512],
                                     start=(ct == 0), stop=(ct == NCT - 1))
                nc.scalar.copy(out=srow[:, cf:cf + 512], in_=st_ps)

            spk = stp.tile([128, SW], f16, tag=f"spk{sc}", name=f"spk{sc}")
            hw_dma(
                out=spk,
                in_=srow[0:1, :].rearrange("one (p w) -> one p w", p=128))
            qpk = stp.tile([128, SW], f16, tag=f"qpk{sc}", name=f"qpk{sc}")
            hw_dma(
                out=qpk,
                in_=srow[32:33, :].rearrange("one (p w) -> one p w", p=128))
            spk_l.append(spk)
            qpk_l.append(qpk)

        for sc in range(NSC):
            spk, qpk = spk_l[sc], qpk_l[sc]
            mu_t = stp.tile([128, SW], f32, tag="mu_t", name="mu_t")
            nc.vector.tensor_scalar_mul(out=mu_t, in0=spk, scalar1=1.0 / C)
            musq = stp.tile([128, SW], f32, tag="musq", name="musq")
            nc.vector.tensor_tensor(out=musq, in0=mu_t, in1=mu_t,
                                    op=OP.mult)
            varq = stp.tile([128, SW], f32, tag="varq", name="varq")
            nc.vector.scalar_tensor_tensor(out=varq, in0=qpk, scalar=1.0 / C,
                                           in1=musq, op0=OP.mult,
                                           op1=OP.subtract)
            sd = stp.tile([128, SW], f32, tag="sd", name="sd")
            nc.scalar.activation(out=sd, in_=varq, func=AF.Sqrt,
                                 bias=eps_c, scale=1.0)
            rs32 = stp.tile([128, SW], f32, tag="rs32", name="rs32")
            nc.vector.reciprocal(out=rs32, in_=sd)
            rsmu = stp.tile([128, 2 * SW], f16, tag=f"rsmu{sc}",
                            name=f"rsmu{sc}")
            nc.vector.tensor_scalar_mul(out=rsmu[:, 0:SW], in0=rs32,
                                        scalar1=1.0)
            nc.vector.scalar_tensor_tensor(out=rsmu[:, SW:2 * SW], in0=spk,
                                           scalar=1.0 / C, in1=rs32,
                                           op0=OP.mult, op1=OP.mult)
            rsmu_l.append(rsmu)
            nc.gpsimd.dma_start(
                out=scr_d[sc:sc + 1, :].rearrange("one (p w) -> one p w",
                                                  p=128),
                in_=rsmu)

        for sc in range(NSC):
            halves = []
            row = scr_d[sc:sc + 1, :]
            for rh in range(2):
                rb = rbp.tile([128, SC], f16, tag=f"rb{rh}", name="rb")
                hw_dma(
                    out=rb,
                    in_=bass.AP(tensor=row.tensor,
                                offset=row.offset + rh * SC,
                                ap=[[0, 128], [1, SC]]))
                halves.append(rb)
            rb_l[sc] = halves

        # per-half state
        qm_r = [None] * NHALF
        xa_r = [None] * NHALF
        prev_hi = {0: -1, 1: -1}

        def windows_dma(h, s, ct, dma):
            # xa window for conv chunk s of half h, channel tile ct
            st = h * LH + s * CH
            dma(
                out=xa_r[h][ct * 64 + s:ct * 64 + s + 49:16, :]
                .rearrange("g (cc w) -> g cc w", cc=GC),
                in_=xa_full[ct][:, st:st + WIN])

        def flush_windows(h, s_hi):
            # inline: only SWDGE windows (HWDGE-ring ones would stall the
            # issuing engine's SEQ between gelus); HWDGE half is deferred
            # to emit_ph2b once the half's gelus are all emitted.
            s_hi = min(s_hi, NCH_H - 1)
            for s in range(prev_hi[h] + 1, s_hi + 1):
                for ct in range(NCT):
                    if (s + ct) % 2 == 1:
                        windows_dma(h, s, ct, nc.gpsimd.dma_start)
            prev_hi[h] = max(prev_hi[h], s_hi)

        def flush_windows_hw(h):
            for s in range(NCH_H):
                for ct in range(NCT):
                    if (s + ct) % 2 == 0:
                        windows_dma(h, s, ct, hw_dma)

        # ============= phase 2: apply + gelu + om (per half) =============
        APW = 1024  # apply op width
        SC_PER_H = LH // SC  # 2

        def emit_apply(sc, lo_off, width):
            # LN apply + gelu for x columns [sc*SC+lo_off, +width)
            rh = (lo_off // SW) // 64
            rb_v = rb_l[sc][rh][:].rearrange("q (p half w) -> q p half w",
                                             p=64, half=2)
            rs_sl = rb_v[:, :, 0, :]
            mu_sl = rb_v[:, :, 1, :]
            lo = sc * SC + lo_off
            pw = width // SW
            p0 = (lo_off // SW) % 64
            for ct in range(NCT):
                xs = x16[ct][:, lo:lo + width].rearrange(
                    "p (a w) -> p a w", a=pw)
                v = vup.tile([128, APW], f16, tag=f"v{ct}", name=f"v{ct}")
                vv = v[:, 0:width].rearrange("p (a w) -> p a w", a=pw)
                nc.vector.tensor_tensor(
                    out=vv, in0=xs, in1=rs_sl[:, p0:p0 + pw, :], op=OP.mult)
                nc.vector.tensor_tensor(
                    out=vv, in0=vv, in1=mu_sl[:, p0:p0 + pw, :],
                    op=OP.subtract)
                nc.scalar.activation(
                    out=xa_full[ct][:, HALO + lo:HALO + lo + width],
                    in_=v[:, 0:width], func=GELU, bias=lnb_c[ct],
                    scale=lnw_c[ct])

        def emit_ph2a(h):
            xa_r[h] = xrp.tile([128, GC * WIN], f16, tag="xa_r",
                               name=f"xa_r{h}")
            for sc_h in range(SC_PER_H):
                sc = h * SC_PER_H + sc_h
                for a2 in range(SC // APW):
                    emit_apply(sc, a2 * APW, APW)
                    cov = sc_h * SC + (a2 + 1) * APW
                    flush_windows(h, (cov - CH - HALO) // CH)
            if h == 0:
                # pre-compute h1's first 512 cols so h0's last window (and
                # thus h0's conv) doesn't wait for h1's front-end. h1
                # recomputes the same values later (idempotent).
                emit_apply(SC_PER_H, 0, 512)
                flush_windows(h, NCH_H - 1)

        def emit_ph2b(h):
            qm_r[h] = qmp.tile([128, 2 * K * LQ], f16, tag="qm_r",
                               name=f"qm_r{h}")
            if h == NHALF - 1:
                flush_windows(h, NCH_H - 1)
            flush_windows_hw(h)
            for gp in range(LH // 512):
                lo5 = h * LH + gp * 512
                om_ps = psum_o.tile([48, 512], f32, tag="om", name="om")
                for ct in range(NCT):
                    nc.tensor.matmul(
                        om_ps, omwT[ct],
                        xa_full[ct][:, HALO + lo5:HALO + lo5 + 512],
                        start=(ct == 0), stop=(ct == NCT - 1))
                om_st = omp.tile([48, 512], f16, tag="om_st", name="om_st")
                if h == 0:
                    # DVE is stalled waiting these scatters anyway; doing the
                    # bias+convert there skips the ACT queue (which is still
                    # busy with h1's gelus) and feeds coeff(h0) ~10us earlier
                    nc.vector.tensor_scalar_add(out=om_st, in0=om_ps,
                                                scalar1=bias48)
                else:
                    nc.scalar.activation(out=om_st, in_=om_ps,
                                         func=AF.Identity, bias=bias48,
                                         scale=1.0)
                for s2 in range(2):
                    s = gp * 2 + s2  # conv chunk within half
                    hw_dma(
                        out=qm_r[h][s:128:16, :]
                        .rearrange("g (o f) -> g o f", o=6),
                        in_=om_st[:, s2 * 256:(s2 + 1) * 256])

        # ================= phase 3: banded conv per half =================
        def emit_ph3(h):
            q_ap = qm_r[h][:, 0:K * LQ]
            m_ap = qm_r[h][:, K * LQ:2 * K * LQ]
            hm_all = cfp.tile([128, ND * K * LQ], f16, tag="hm",
                              name=f"hm{h}")
            r2t = cfp.tile([128, K * LQ], f16, tag="r2t", name="r2t")
            for i in range(ND):
                d = D_LO + i
                sl = hm_all[:, i * K * LQ:(i + 1) * K * LQ]
                nc.vector.tensor_scalar_add(out=sl, in0=q_ap,
                                            scalar1=float(1 - d))
                nc.vector.tensor_scalar(out=r2t, in0=q_ap,
                                        scalar1=float(d + 1), scalar2=-1.0,
                                        op0=OP.subtract, op1=OP.mult)
                nc.vector.tensor_tensor(out=sl, in0=sl, in1=r2t, op=OP.min)
                nc.vector.tensor_scalar_max(out=sl, in0=sl, scalar1=0.0)
            m_rep = bass.AP(tensor=m_ap.tensor, offset=m_ap.offset,
                            ap=[m_ap.ap[0], [0, ND], [1, K * LQ]])
            nc.vector.tensor_tensor(
                out=hm_all[:].rearrange("p (d f) -> p d f", d=ND),
                in0=hm_all[:].rearrange("p (d f) -> p d f", d=ND),
                in1=m_rep, op=OP.mult)
            cd_all = cfp.tile([128, ND * LQ], f16, tag="cd", name=f"cd{h}")
            hm_v = hm_all[:].rearrange("p (d k f) -> p d k f", d=ND, k=K)
            cd_v = cd_all[:].rearrange("p (d f) -> p d f", d=ND)
            nc.vector.tensor_tensor(out=cd_v, in0=hm_v[:, :, 0, :],
                                    in1=hm_v[:, :, 1, :], op=OP.add)
            nc.vector.tensor_tensor(out=cd_v, in0=cd_v,
                                    in1=hm_v[:, :, 2, :], op=OP.add)

            xa_r_v = xa_r[h][:].rearrange("p (c wn) -> p c wn", c=GC)
            out_t = out_d[:]
            ost_all = outp.tile([128, GC * LQ], f16, tag="ost_all",
                                name=f"ost_all{h}")
            for cp in range(NCP):
                tmps = []
                for i in range(ND):
                    d = D_LO + i
                    tmp = tmpp.tile([128, CPG * LQ], f16, tag=f"tmp{i}",
                                    name=f"tmp{i}")
                    cdi = cd_all[:, i * LQ:(i + 1) * LQ]
                    cb = bass.AP(tensor=cdi.tensor, offset=cdi.offset,
                                 ap=[cdi.ap[0], [0, CPG], [1, LQ]])
                    # Pool is idle in the tail; let it take a slice of the
                    # last half's products to shorten the DVE-bound tail.
                    peng = (nc.gpsimd if (cp >= 2 + 5 * (1 - h) and i == 4)
                            else nc.vector)
                    peng.tensor_tensor(
                        out=tmp[:].rearrange("p (c f) -> p c f", c=CPG),
                        in0=xa_r_v[:, cp * CPG:(cp + 1) * CPG,
                                   HALO + d:HALO + d + LQ],
                        in1=cb, op=OP.mult)
                    tmps.append(tmp)
                o0 = cp * CPG * LQ
                if cp in DSUM_DVE_CP:
                    nc.vector.tensor_tensor(out=tmps[0], in0=tmps[0],
                                            in1=tmps[1], op=OP.add)
                    nc.vector.tensor_tensor(out=tmps[2], in0=tmps[2],
                                            in1=tmps[3], op=OP.add)
                    nc.vector.tensor_tensor(out=tmps[0], in0=tmps[0],
                                            in1=tmps[2], op=OP.add)
                    nc.vector.tensor_tensor(
                        out=ost_all[:, o0:o0 + CPG * LQ], in0=tmps[0],
                        in1=tmps[4], op=OP.add)
                else:
                    for jp in range(CPG // 2):
                        acc = psum_a.tile([128, 2 * LQ], f32, tag="acc",
                                          name="acc")
                        for jj in range(2):
                            j = jp * 2 + jj
                            for n_i in range(ND):
                                nc.tensor.matmul(
                                    acc[:, jj * LQ:(jj + 1) * LQ], id16,
                                    tmps[n_i][:, j * LQ:(j + 1) * LQ],
                                    start=(n_i == 0), stop=(n_i == ND - 1))
                        oj = o0 + jp * 2 * LQ
                        if (cp, jp) in OST_POOL_JP:
                            nc.gpsimd.tensor_copy(
                                out=ost_all[:, oj:oj + 2 * LQ], in_=acc)
                        else:
                            nc.scalar.copy(
                                out=ost_all[:, oj:oj + 2 * LQ], in_=acc)
            for g in range(G):
                hw_dma(
                    out=bass.AP(
                        tensor=out_t.tensor,
                        offset=out_t.offset + g * GC * L + h * LH,
                        ap=[[CH, NCH_H], [L, GC], [1, LQ]]),
                    in_=ost_all[g * 16:g * 16 + 16, :]
                    .rearrange("s (c f) -> s c f", c=GC))

        # driver: front-ends first (conv overlaps them via per-engine
        # in-order streams)
        emit_ph2a(0)
        emit_ph2a(1)
        emit_ph2b(0)
        emit_ph2b(1)
        emit_ph3(0)
        emit_ph3(1)

    return nc


def _prep_params(ln_w, ln_b, om_w, om_b):
    cst32 = np.zeros((128, 6), np.float32)
    cst32[:, 0] = ln_w[0:128]
    cst32[:, 1] = ln_w[128:256]
    cst32[:, 2] = ln_b[0:128]
    cst32[:, 3] = ln_b[128:256]
    cst32[:, 4] = 1.0
    # bias48 in (g, qm, k) order with conv grid fold for qm=0 (offsets)
    for g in range(G):
        for k in range(K):
            cst32[g * 6 + k, 5] = om_b[g * K + k] + (k - 1.0)
            cst32[g * 6 + 3 + k, 5] = om_b[G * K + g * K + k]
    cst16 = np.zeros((128, 226), np.float16)
    omwT = om_w.T.astype(np.float16)          # [C, 48] original row order
    perm = np.zeros(48, np.int64)
    for g in range(G):
        for k in range(K):
            perm[g * 6 + k] = g * K + k
            perm[g * 6 + 3 + k] = G * K + g * K + k
    omwTp = omwT[:, perm]                      # columns in (g,qm,k) order
    cst16[:, 0:48] = omwTp[0:128]
    cst16[:, 48:96] = omwTp[128:256]
    cst16[:, 96:224] = np.eye(128, dtype=np.float16)
    cst16[:, 224] = 1.0
    return {"cst32": cst32, "cst16": cst16}


def kernel(x, ln_w, ln_b, om_w, om_b):
    _install_patch()
    from concourse.bass_utils import run_bass_kernel_spmd

    if "nc" not in _cache:
        _cache["nc"] = _build_module()
    nc = _cache["nc"]

    x = np.ascontiguousarray(np.asarray(x, dtype=np.float32))
    params = _prep_params(np.asarray(ln_w, np.float32),
                          np.asarray(ln_b, np.float32),
                          np.asarray(om_w, np.float32),
                          np.asarray(om_b, np.float32))
    in_maps = [dict(params, x=x[n]) for n in range(N)]
    res = run_bass_kernel_spmd(nc, in_maps, core_ids=list(range(N)))
    return np.stack([res.results[n]["out"] for n in range(N)],
                    axis=0).astype(np.float32)


def _prep_inputs(inputs):
    x = np.ascontiguousarray(np.asarray(inputs["x"], dtype=np.float32))
    params = _prep_params(np.asarray(inputs["ln_w"], np.float32),
                          np.asarray(inputs["ln_b"], np.float32),
                          np.asarray(inputs["om_w"], np.float32),
                          np.asarray(inputs["om_b"], np.float32))
    return [dict(params, x=x[n]) for n in range(N)]


def run_traced(inputs):
    _install_patch()
    from concourse.bass_utils import run_bass_kernel_spmd
    if "nc" not in _cache:
        _cache["nc"] = _build_module()
    return run_bass_kernel_spmd(_cache["nc"], _prep_inputs(inputs),
                                core_ids=list(range(N)), trace=True)
